# revision 1
# baseline (speedup 1.0000x reference)
"""Multi-head attention (B=4, S=2048, D=1024, H=16, dk=dv=64) on 8 TRN2 cores.

Sharding: core = (batch b, head-group g): data-parallel over batch (4) x
tensor-parallel over heads (2 groups of 8). Each core computes its batch's
Q/K/V projections for its 8 heads, attention, and a partial output
projection over its heads' rows of Wo. The host sums the two partial
outputs per batch.

Per-core kernel (matmul inputs float32r = TF32-like, fp32 accumulate):
  V phase:  xT streamed in s-tiles of 512; V for all 8 heads projected into
            a [s, head, 64+1] layout whose per-head ones column later yields
            softmax denominators for free.
  pair loop (4 head pairs): Q/K projected pair-packed (two heads' 64 dk dims
            on partitions 0:64 / 64:128, xT re-streamed), then attention:
            per 512-q-block, both heads' scores land in one [128,1024] PSUM
            tile via matmuls contracting disjoint row-groups (concurrent on
            the PE), one ACT exp (scale=1/8 folded in) covers both, AV
            accumulates per head over 16 s-chunks; row 64 of the AV PSUM is
            the softmax denominator. Normalize = DVE reciprocal + GPSIMD
            partition broadcast + DVE multiply.
  out phase: output projection from the normalized [hv, s] head layout.
"""

import numpy as np

import concourse.bacc as bacc
import concourse.tile as tile
import concourse.mybir as mybir
from concourse.bass_utils import run_bass_kernel_spmd

F32 = mybir.dt.float32
F32R = mybir.dt.float32r
EXP = mybir.ActivationFunctionType.Exp

P = 128
S = 2048
D = 1024
DK = 64
HPC = 8            # heads per core
NSC = S // P       # 16 s-chunks of 128
NST = 4            # s-tiles of 512
STW = S // NST     # 512
NDC = D // P       # 8 d_model chunks
NPAIR = HPC // 2   # 4 head pairs
NQB = S // STW     # 4 q-blocks of 512 in attention
SCALE = 1.0 / np.sqrt(DK)


def build_kernel():
    nc = bacc.Bacc("TRN2", target_bir_lowering=False, debug=False)

    xt_d = nc.dram_tensor("xt", [D, S], F32R, kind="ExternalInput")
    wq_d = nc.dram_tensor("wq", [NPAIR, D, P], F32R, kind="ExternalInput")
    wk_d = nc.dram_tensor("wk", [NPAIR, D, P], F32R, kind="ExternalInput")
    wv_d = nc.dram_tensor("wv", [D, HPC * DK], F32R, kind="ExternalInput")
    wo_d = nc.dram_tensor("wo", [HPC * DK, D], F32R, kind="ExternalInput")
    ones_d = nc.dram_tensor("ones", [P, P], F32R, kind="ExternalInput")
    out_d = nc.dram_tensor("out", [S, D], F32, kind="ExternalOutput")

    xt_ap = xt_d.ap().rearrange("(dc p) s -> p dc s", p=P)

    with tile.TileContext(nc) as tc:
        with tc.tile_pool(name="persist", bufs=1) as persist, \
             tc.tile_pool(name="xtp", bufs=2) as xtp, \
             tc.tile_pool(name="stage", bufs=4) as stage:
            # V with ones column per head: [128 s, sc, head, 64+1]
            v520 = persist.tile([P, NSC, HPC, DK + 1], F32R, tag="v520")
            # normalized heads, [hv, s]: chunk ci = heads (2ci, 2ci+1)
            hn = persist.tile([P, NPAIR, S], F32R, tag="hn")
            wo_sb = persist.tile([P, NPAIR, D], F32R, tag="wo")
            ones_sb = persist.tile([P, HPC], F32R, tag="ones_sb")

            # -------- per-pair: QK projection + attention --------
            # Pair 0's first pass also projects V (shared xT stream), with
            # the pair-0 Q/K matmuls issued first so the first scores/exp
            # start as early as possible.
            with tc.tile_pool(name="wvp", bufs=1) as wvp, \
                 tc.tile_pool(name="qkpool", bufs=2) as qkpool, \
                 tc.tile_pool(name="wqkp", bufs=2) as wqkp, \
                 tc.tile_pool(name="expp", bufs=2) as expp, \
                 tc.tile_pool(name="smallp", bufs=2) as smallp, \
                 tc.tile_pool(name="bcsb", bufs=2) as bcsb, \
                 tc.tile_pool(name="qkps", bufs=2, space="PSUM") as qkps, \
                 tc.tile_pool(name="scps", bufs=2, space="PSUM") as scps, \
                 tc.tile_pool(name="avps", bufs=1, space="PSUM") as avps:
                vps = qkps  # shared double-buffered [128, 512] psum tag
                wv_sb = wvp.tile([P, NDC, HPC * DK], F32R, tag="wv")
                for pr in range(NPAIR):
                    wqp = wqkp.tile([P, NDC, P], F32R, tag="wqp")
                    wkp = wqkp.tile([P, NDC, P], F32R, tag="wkp")
                    nc.sync.dma_start(
                        wqp[:],
                        wq_d.ap()[pr].rearrange("(dc p) c -> p dc c", p=P))
                    nc.sync.dma_start(
                        wkp[:],
                        wk_d.ap()[pr].rearrange("(dc p) c -> p dc c", p=P))
                    if pr == 0:
                        # needed from the first pass, but after pair-0 Q/K
                        nc.sync.dma_start(ones_sb[:], ones_d.ap()[:, 0:HPC])
                        nc.sync.dma_start(
                            wv_sb[:],
                            wv_d.ap().rearrange("(dc p) c -> p dc c", p=P))
                    elif pr == 1:
                        # only needed by the output projection at the end
                        nc.sync.dma_start(
                            wo_sb[:],
                            wo_d.ap().rearrange("(ci p) d -> p ci d", p=P))
                    qtp = qkpool.tile([P, S], F32R, tag="qt")
                    ktp = qkpool.tile([P, S], F32R, tag="kt")

                    def out_group(sc, dmh):
                        # one output-projection psum group ([128, 512] out
                        # rows sc, cols dmh-half); reuses the qkp PSUM banks
                        ps = qkps.tile([P, D // 2], F32, tag="qkp",
                                       name="ops")
                        for ci in range(NPAIR):
                            nc.tensor.matmul(
                                ps[:], hn[:, ci, sc * P:(sc + 1) * P],
                                wo_sb[:, ci, dmh * 512:(dmh + 1) * 512],
                                start=(ci == 0), stop=(ci == NPAIR - 1))
                        osb = stage.tile([P, D // 2], F32,
                                         tag="ostage", name="osb")
                        nc.vector.tensor_copy(osb[:], ps[:])
                        nc.sync.dma_start(
                            out_d.ap()[sc * P:(sc + 1) * P,
                                       dmh * 512:(dmh + 1) * 512],
                            osb[:])

                    def attn_chunk(pr, qb, sc_lo, sc_hi, av, trail=None,
                                   qtp=qtp, ktp=ktp):
                        q0 = qb * STW
                        for sc in range(sc_lo, sc_hi):
                            scp = scps.tile([P, 2 * STW], F32, tag="scp",
                                            name="scp")
                            for j in range(2):
                                nc.tensor.matmul(
                                    scp[:, j * STW:(j + 1) * STW],
                                    ktp[j * DK:(j + 1) * DK,
                                        sc * P:(sc + 1) * P],
                                    qtp[j * DK:(j + 1) * DK, q0:q0 + STW],
                                    start=True, stop=True)
                            ex = expp.tile([P, 2 * STW], F32R, tag="exp",
                                           name="ex")
                            nc.scalar.activation(
                                ex[:], scp[:], EXP, scale=float(SCALE))
                            for j in range(2):
                                nc.tensor.matmul(
                                    av[j][:], v520[:, sc, 2 * pr + j, :],
                                    ex[:, j * STW:(j + 1) * STW],
                                    start=(sc == 0), stop=(sc == NSC - 1),
                                    skip_group_check=True)
                            if trail and sc % 2 == 1:
                                out_group(*trail.pop(0))

                    def attn_norm(pr, qb, av):
                        q0 = qb * STW
                        for j in range(2):
                            # copy PSUM->SBUF first so the AV bank frees
                            # immediately; normalize off the critical path
                            avs = bcsb.tile([DK + 1, STW], F32, tag="avs",
                                            name="avs")
                            nc.vector.tensor_copy(avs[:], av[j][:])
                            rec = smallp.tile([1, STW], F32R, tag="rec",
                                              name="rec")
                            with nc.allow_low_precision(
                                    reason="softmax recip feeds fp32r mm"):
                                nc.vector.reciprocal(
                                    rec[:], avs[DK:DK + 1, :])
                            bcs = bcsb.tile([DK, STW], F32R, tag="bcs",
                                            name="bcs")
                            nc.gpsimd.partition_broadcast(
                                bcs[:], rec[:], channels=DK)
                            nc.vector.tensor_mul(
                                hn[j * DK:(j + 1) * DK, pr, q0:q0 + STW],
                                avs[0:DK, :], bcs[:])

                    def new_av():
                        return [avps.tile([DK + 1, STW], F32, tag=f"av{j}",
                                          name=f"av{j}")
                                for j in range(2)]

                    av0 = new_av() if pr == 0 else None
                    for st in range(NST):
                        xts = xtp.tile([P, NDC, STW], F32R, tag="xts")
                        for dh in range(0, NDC, 2):
                            nc.sync.dma_start(
                                xts[:, dh:dh + 2, :],
                                xt_ap[:, dh:dh + 2,
                                      st * STW:(st + 1) * STW])
                        for w_sb, dst in ((wqp, qtp), (wkp, ktp)):
                            ps = qkps.tile([P, STW], F32, tag="qkp")
                            for dc in range(NDC):
                                nc.tensor.matmul(
                                    ps[:], w_sb[:, dc, :], xts[:, dc, :],
                                    start=(dc == 0), stop=(dc == NDC - 1))
                            nc.vector.tensor_copy(
                                dst[:, st * STW:(st + 1) * STW], ps[:])
                        if pr < 2:
                            # V projection rides the first two pairs' xT
                            # streams, half the heads each (N=256 keeps the
                            # fp32r full-rate >=256 threshold); pair 0 only
                            # needs heads 0-1's V for its own attention.
                            h0 = pr * (HPC // 2)
                            c0 = h0 * DK
                            for scl in range(STW // P):
                                sc = st * (STW // P) + scl
                                ps = vps.tile([P, HPC * DK], F32, tag="qkp")
                                for dc in range(NDC):
                                    nc.tensor.matmul(
                                        ps[:, 0:HPC * DK // 2],
                                        xts[:, dc, scl * P:(scl + 1) * P],
                                        wv_sb[:, dc, c0:c0 + HPC * DK // 2],
                                        start=(dc == 0), stop=(dc == NDC - 1))
                                nc.vector.tensor_copy(
                                    v520[:, sc, h0:h0 + HPC // 2, 0:DK],
                                    ps[:, 0:HPC * DK // 2].rearrange(
                                        "p (h v) -> p h v", v=DK))
                                nc.vector.tensor_copy(
                                    v520[:, sc, h0:h0 + HPC // 2,
                                         DK:DK + 1],
                                    ones_sb[:, h0:h0 + HPC // 2, None])
                        if pr == 0:
                            # pair-0 q-block 0 starts as soon as this st's
                            # K/V chunks exist (queries 0:512 are st 0)
                            attn_chunk(0, 0, st * 4, (st + 1) * 4, av0)

                    # attention for this pair, per 512-wide q-block. For the
                    # last pair, q-block qb-1 is complete once norm(qb-1)
                    # ran, so its output-projection groups interleave into
                    # attention of qb (one group per two s-chunks).
                    if pr == 0:
                        attn_norm(0, 0, av0)
                    for qb in range(1 if pr == 0 else 0, NQB):
                        av = new_av()
                        trail = None
                        if pr == NPAIR - 1 and qb > 0:
                            trail = [(sc, dmh)
                                     for sc in range((qb - 1) * 4, qb * 4)
                                     for dmh in range(2)]
                        attn_chunk(pr, qb, 0, NSC, av, trail=trail)
                        attn_norm(pr, qb, av)
                    if pr == NPAIR - 1:
                        for sc in range((NQB - 1) * 4, NQB * 4):
                            for dmh in range(2):
                                out_group(sc, dmh)

    nc.compile()
    return nc


_NC_CACHE = None


def _get_nc():
    global _NC_CACHE
    if _NC_CACHE is None:
        _NC_CACHE = build_kernel()
    return _NC_CACHE


def kernel(x, Wq, Wk, Wv, Wo):
    x = np.asarray(x, dtype=np.float32)
    Wq = np.asarray(Wq, dtype=np.float32)
    Wk = np.asarray(Wk, dtype=np.float32)
    Wv = np.asarray(Wv, dtype=np.float32)
    Wo = np.asarray(Wo, dtype=np.float32)
    B = x.shape[0]
    ones = np.ones((P, P), dtype=np.float32)

    in_maps = []
    for core in range(8):
        b, g = divmod(core, 2)
        hs = g * HPC
        xt = np.ascontiguousarray(x[b].T)
        wq = np.stack([
            np.concatenate([Wq[hs + 2 * p], Wq[hs + 2 * p + 1]], axis=1)
            for p in range(NPAIR)])
        wk = np.stack([
            np.concatenate([Wk[hs + 2 * p], Wk[hs + 2 * p + 1]], axis=1)
            for p in range(NPAIR)])
        wv = np.concatenate([Wv[hs + h] for h in range(HPC)], axis=1)
        wo = np.ascontiguousarray(Wo[hs * DK:(hs + HPC) * DK, :])
        in_maps.append({"xt": xt, "wq": wq, "wk": wk, "wv": wv, "wo": wo,
                        "ones": ones})

    nc = _get_nc()
    res = run_bass_kernel_spmd(nc, in_maps, core_ids=list(range(8))).results

    out = np.empty((B, S, D), dtype=np.float32)
    for b in range(B):
        out[b] = res[2 * b]["out"] + res[2 * b + 1]["out"]
    return out



# revision 6
# speedup vs baseline: 1.0172x; 1.0172x over previous
"""Multi-head attention (B=4, S=2048, D=1024, H=16, dk=dv=64) on 8 TRN2 cores.

Sharding: core = (batch b, head-group g): data-parallel over batch (4) x
tensor-parallel over heads (2 groups of 8). Each core computes its batch's
Q/K/V projections for its 8 heads, attention, and a partial output
projection over its heads' rows of Wo. The host sums the two partial
outputs per batch.

Per-core kernel, all matmul operands bf16 (fp32 PSUM accumulate):
  xT is DMA'd once into SBUF (bf16, 32KB/partition) and reused by all
  projections. Heads are processed in 4 pairs; per pair Q/K are projected
  pair-packed ([2x64 dk, S]); V rides ahead one pair as PE filler work.

  Attention is a flat software pipeline over (pair, query-block, key-chunk)
  steps paced by the ACT engine (exp of the 128x1024 score tile is the
  global floor at ~266us/core). Per key chunk both heads' scores land in
  one [128, 1024] PSUM tile and one exp covers both. The AV matmul is
  query-stationary: lhsT = exp-tile slice [128 s, 128 q], rhs =
  [V_h | ones] [128 s, 65] -> PSUM [128 q, 65] accumulated over 16 key
  chunks; column 64 is the softmax denominator. This costs 65 rows/matmul
  instead of 512 for the value-stationary form. After a block's 16 chunks
  the accumulator is copied to SBUF (freeing the PSUM bank for the next
  block), normalized with a DVE reciprocal + per-partition-scalar multiply,
  and PE-transposed back to [hv, q] for the output projection. Scores/exp
  are emitted one step ahead of AV so the ACT queue never starves;
  projection / V / output-projection matmul groups are pumped into the PE
  queue as filler between steps.
"""

import collections

import numpy as np

import concourse.bacc as bacc
import concourse.tile as tile
import concourse.mybir as mybir
from concourse.bass_utils import run_bass_kernel_spmd

F32 = mybir.dt.float32
BF16 = mybir.dt.bfloat16
EXP = mybir.ActivationFunctionType.Exp

P = 128
S = 2048
D = 1024
DK = 64
HPC = 8            # heads per core
NSC = S // P       # 16 key chunks of 128
NST = 4            # s-tiles of 512
STW = S // NST     # 512
NDC = D // P       # 8 d_model chunks
NPAIR = HPC // 2   # 4 head pairs
NQB = S // STW     # 4 query blocks of 512
NQC = STW // P     # 4 query chunks of 128 per block
SCALE = 1.0 / np.sqrt(DK)


def build_kernel():
    nc = bacc.Bacc("TRN2", target_bir_lowering=False, debug=False)

    xt_d = nc.dram_tensor("xt", [D, S], BF16, kind="ExternalInput")
    wq_d = nc.dram_tensor("wq", [NPAIR, D, P], BF16, kind="ExternalInput")
    wk_d = nc.dram_tensor("wk", [NPAIR, D, P], BF16, kind="ExternalInput")
    wv_d = nc.dram_tensor("wv", [D, HPC * DK], BF16, kind="ExternalInput")
    wo_d = nc.dram_tensor("wo", [HPC * DK, D], BF16, kind="ExternalInput")
    id_d = nc.dram_tensor("ident", [P, P], BF16, kind="ExternalInput")
    out_d = nc.dram_tensor("out", [S, D], F32, kind="ExternalOutput")

    xt_ap = xt_d.ap().rearrange("(dc p) s -> p dc s", p=P)

    with tile.TileContext(nc) as tc:
        with tc.tile_pool(name="persist", bufs=1) as persist, \
             tc.tile_pool(name="qkpool", bufs=2) as qkpool, \
             tc.tile_pool(name="wqkp", bufs=2) as wqkp, \
             tc.tile_pool(name="expp", bufs=3) as expp, \
             tc.tile_pool(name="avsbp", bufs=2) as avsbp, \
             tc.tile_pool(name="htsp", bufs=3) as htsp, \
             tc.tile_pool(name="recp", bufs=4) as recp, \
             tc.tile_pool(name="stage", bufs=2) as stage, \
             tc.tile_pool(name="scps", bufs=2, space="PSUM") as scps, \
             tc.tile_pool(name="avps", bufs=1, space="PSUM") as avps, \
             tc.tile_pool(name="qkps", bufs=2, space="PSUM") as qkps:

            xts = persist.tile([P, NDC, S], BF16, tag="xts")
            v520 = persist.tile([P, NSC, HPC, DK + 1], BF16, tag="v520")
            hn = persist.tile([P, NPAIR, S], BF16, tag="hn")
            wo_sb = persist.tile([P, NPAIR, D], BF16, tag="wo")
            wv_sb = persist.tile([P, NDC, HPC * DK], BF16, tag="wv")
            id_sb = persist.tile([P, P], BF16, tag="id")

            # ---------------- prologue DMAs ----------------
            wqk_tiles = {}

            def fetch_wqk(pr):
                wqp = wqkp.tile([P, NDC, P], BF16, tag="wqp", name=f"wqp{pr}")
                wkp = wqkp.tile([P, NDC, P], BF16, tag="wkp", name=f"wkp{pr}")
                nc.sync.dma_start(
                    wqp[:], wq_d.ap()[pr].rearrange("(dc p) c -> p dc c", p=P))
                nc.sync.dma_start(
                    wkp[:], wk_d.ap()[pr].rearrange("(dc p) c -> p dc c", p=P))
                wqk_tiles[pr] = (wqp, wkp)

            fetch_wqk(0)
            # xT in 16 chunks; query-block-0 columns first so pair-0 can start
            for qtr in range(4):
                for dh in range(NDC // 2):
                    nc.sync.dma_start(
                        xts[:, 2 * dh:2 * dh + 2, qtr * 512:(qtr + 1) * 512],
                        xt_ap[:, 2 * dh:2 * dh + 2, qtr * 512:(qtr + 1) * 512])
            nc.sync.dma_start(
                wv_sb[:], wv_d.ap().rearrange("(dc p) c -> p dc c", p=P))
            nc.sync.dma_start(id_sb[:], id_d.ap())
            nc.sync.dma_start(
                wo_sb[:], wo_d.ap().rearrange("(ci p) d -> p ci d", p=P))
            nc.vector.memset(v520[:, :, :, DK:DK + 1], 1.0)

            qk_tiles = {0: (qkpool.tile([P, S], BF16, tag="qt", name="qt0"),
                            qkpool.tile([P, S], BF16, tag="kt", name="kt0"))}

            # ---------------- helpers ----------------
            def qk_group(pr, which, st, sub_sc=False):
                """Project qtp/ktp columns st*512:(st+1)*512 for pair pr."""
                w_sb = wqk_tiles[pr][0 if which == "q" else 1]
                dst = qk_tiles[pr][0 if which == "q" else 1]
                ps = qkps.tile([P, STW], F32, tag="qkp", name=f"{which}{pr}{st}")
                if sub_sc:
                    # key-chunk granular psum + copies (fast prologue start)
                    for scl in range(4):
                        for dc in range(NDC):
                            nc.tensor.matmul(
                                ps[:, scl * P:(scl + 1) * P], w_sb[:, dc, :],
                                xts[:, dc, (st * 4 + scl) * P:(st * 4 + scl + 1) * P],
                                start=(dc == 0), stop=(dc == NDC - 1))
                        nc.vector.tensor_copy(
                            dst[:, (st * 4 + scl) * P:(st * 4 + scl + 1) * P],
                            ps[:, scl * P:(scl + 1) * P])
                else:
                    for dc in range(NDC):
                        nc.tensor.matmul(
                            ps[:], w_sb[:, dc, :],
                            xts[:, dc, st * STW:(st + 1) * STW],
                            start=(dc == 0), stop=(dc == NDC - 1))
                    nc.vector.tensor_copy(
                        dst[:, st * STW:(st + 1) * STW], ps[:])

            def v_group(pr, sc):
                """Project V for pair pr's two heads, key chunk sc."""
                ps = qkps.tile([P, STW], F32, tag="qkp", name=f"v{pr}{sc}")
                for dc in range(NDC):
                    nc.tensor.matmul(
                        ps[:, 0:P], xts[:, dc, sc * P:(sc + 1) * P],
                        wv_sb[:, dc, pr * P:(pr + 1) * P],
                        start=(dc == 0), stop=(dc == NDC - 1))
                nc.vector.tensor_copy(
                    v520[:, sc, 2 * pr:2 * pr + 2, 0:DK],
                    ps[:, 0:P].rearrange("p (h v) -> p h v", v=DK))

            def out_group(sc_out, dmh):
                ps = qkps.tile([P, STW], F32, tag="qkp", name=f"o{sc_out}{dmh}")
                for ci in range(NPAIR):
                    nc.tensor.matmul(
                        ps[:], hn[:, ci, sc_out * P:(sc_out + 1) * P],
                        wo_sb[:, ci, dmh * 512:(dmh + 1) * 512],
                        start=(ci == 0), stop=(ci == NPAIR - 1))
                osb = stage.tile([P, STW], F32, tag="ostage", name="osb")
                nc.vector.tensor_copy(osb[:], ps[:])
                nc.sync.dma_start(
                    out_d.ap()[sc_out * P:(sc_out + 1) * P,
                               dmh * 512:(dmh + 1) * 512],
                    osb[:])

            filler = collections.deque()

            def pump(n=1):
                for _ in range(n):
                    if filler:
                        filler.popleft()()

            def scores_exp(pr, qb, sc):
                qtp, ktp = qk_tiles[pr]
                q0 = qb * STW
                scp = scps.tile([P, 2 * STW], F32, tag="scp", name="scp")
                for j in range(2):
                    nc.tensor.matmul(
                        scp[:, j * STW:(j + 1) * STW],
                        ktp[j * DK:(j + 1) * DK, sc * P:(sc + 1) * P],
                        qtp[j * DK:(j + 1) * DK, q0:q0 + STW],
                        start=True, stop=True)
                ex = expp.tile([P, 2 * STW], BF16, tag="ex", name="ex")
                nc.scalar.activation(ex[:], scp[:], EXP, scale=float(SCALE))
                return ex

            av_cur = [None]

            def av_mms(pr, qb, sc, ex):
                if sc == 0:
                    av_cur[0] = [avps.tile([P, NQC * 65], F32, tag=f"av{j}",
                                           name=f"av{j}_{pr}{qb}")
                                 for j in range(2)]
                av = av_cur[0]
                for j in range(2):
                    for qc in range(NQC):
                        nc.tensor.matmul(
                            av[j][:, qc * 65:(qc + 1) * 65],
                            ex[:, j * STW + qc * P:j * STW + (qc + 1) * P],
                            v520[:, sc, 2 * pr + j, :],
                            # start=True zeroes the whole 2KB psum bank, so
                            # only the first of the 4 interleaved qc groups
                            # starts; the rest overwrite-on-first-write via
                            # the bank-wide pending-zero.
                            start=(sc == 0 and qc == 0),
                            stop=(sc == NSC - 1),
                            skip_group_check=True)
                if sc == NSC - 1:
                    avsb = avsbp.tile([P, 2, NQC * 65], F32, tag="avsb",
                                      name=f"avsb{pr}{qb}")
                    for j in range(2):
                        nc.vector.tensor_copy(avsb[:, j, :], av[j][:])
                    return avsb
                return None

            def norm_qc(pr, qb, avsb, qc):
                """Normalize + transpose one 128-query chunk of both heads."""
                hts = htsp.tile([P, P], BF16, tag="hts", name=f"h{pr}{qb}{qc}")
                for j in range(2):
                    rec = recp.tile([P, 1], F32, tag="rec", name="rec")
                    nc.vector.reciprocal(
                        rec[:], avsb[:, j, qc * 65 + DK:qc * 65 + DK + 1])
                    nc.vector.tensor_scalar_mul(
                        hts[:, j * DK:(j + 1) * DK],
                        avsb[:, j, qc * 65:qc * 65 + DK], rec[:])
                tp = qkps.tile([P, STW], F32, tag="qkp", name=f"t{pr}{qb}{qc}")
                tpb = tp[:, 0:P // 2].bitcast(BF16)
                nc.tensor.transpose(tpb, hts[:], id_sb[:])
                nc.vector.tensor_copy(
                    hn[:, pr, (qb * NQC + qc) * P:(qb * NQC + qc + 1) * P],
                    tpb)

            norm_pend = [None]   # [pr, qb, avsb, next_qc]

            def norm_slot():
                if norm_pend[0] is None:
                    return
                npr, nqb, avsb, qc = norm_pend[0]
                norm_qc(npr, nqb, avsb, qc)
                if npr == NPAIR - 1:
                    so = nqb * NQC + qc
                    filler.append(lambda so=so: out_group(so, 0))
                    filler.append(lambda so=so: out_group(so, 1))
                if qc == NQC - 1:
                    norm_pend[0] = None
                else:
                    norm_pend[0] = [npr, nqb, avsb, qc + 1]

            # ---------------- pair 0: projections + qb0 interleaved --------
            fetch_wqk(1)
            exs = {}
            for st in range(NST):
                qk_group(0, "q", st, sub_sc=(st == 0))
                qk_group(0, "k", st, sub_sc=(st == 0))
                for scl in range(4):
                    v_group(0, st * 4 + scl)
                for scl in range(4):
                    sc = st * 4 + scl
                    exs[(0, 0, sc)] = scores_exp(0, 0, sc)
                    if sc >= 1:
                        avsb = av_mms(0, 0, sc - 1, exs.pop((0, 0, sc - 1)))

            # ---------------- flat attention pipeline ----------------
            rest = [(pr, qb, sc)
                    for pr in range(NPAIR)
                    for qb in range(NQB)
                    for sc in range(NSC)][NSC:]          # from (0, 1, 0)
            av_list = [(0, 0, NSC - 1)] + rest[:-1]       # one step behind

            for t, cur in enumerate(rest):
                apr, aqb, asc = av_list[t]
                if (t == 0 or (aqb, asc) == (0, 0)) and apr + 1 < NPAIR:
                    # stage next pair's projections as PE filler work
                    nxt = apr + 1
                    qk_tiles[nxt] = (
                        qkpool.tile([P, S], BF16, tag="qt", name=f"qt{nxt}"),
                        qkpool.tile([P, S], BF16, tag="kt", name=f"kt{nxt}"))
                    if nxt + 1 < NPAIR:
                        fetch_wqk(nxt + 1)
                    for st in range(NST):
                        filler.append(lambda st=st, n=nxt: qk_group(n, "q", st))
                        filler.append(lambda st=st, n=nxt: qk_group(n, "k", st))
                    for sc in range(NSC):
                        filler.append(lambda sc=sc, n=nxt: v_group(n, sc))
                exs[cur] = scores_exp(*cur)
                norm_slot()
                avsb = av_mms(*av_list[t], exs.pop(av_list[t]))
                if avsb is not None:
                    norm_pend[0] = [apr, aqb, avsb, 0]
                pump(1)

            # ---------------- epilogue ----------------
            last = rest[-1]
            avsb = av_mms(*last, exs.pop(last))
            for qc in range(NQC):
                norm_qc(last[0], last[1], avsb, qc)
                out_group(last[1] * NQC + qc, 0)
                out_group(last[1] * NQC + qc, 1)
            while filler:
                pump(1)

    nc.compile()
    return nc


_NC_CACHE = None


def _get_nc():
    global _NC_CACHE
    if _NC_CACHE is None:
        _NC_CACHE = build_kernel()
    return _NC_CACHE


def kernel(x, Wq, Wk, Wv, Wo):
    import ml_dtypes
    bf16 = ml_dtypes.bfloat16

    x = np.asarray(x, dtype=np.float32)
    Wq = np.asarray(Wq, dtype=np.float32)
    Wk = np.asarray(Wk, dtype=np.float32)
    Wv = np.asarray(Wv, dtype=np.float32)
    Wo = np.asarray(Wo, dtype=np.float32)
    B = x.shape[0]
    ident = np.eye(P, dtype=bf16)

    in_maps = []
    for core in range(8):
        b, g = divmod(core, 2)
        hs = g * HPC
        xt = np.ascontiguousarray(x[b].T).astype(bf16)
        wq = np.stack([
            np.concatenate([Wq[hs + 2 * p], Wq[hs + 2 * p + 1]], axis=1)
            for p in range(NPAIR)]).astype(bf16)
        wk = np.stack([
            np.concatenate([Wk[hs + 2 * p], Wk[hs + 2 * p + 1]], axis=1)
            for p in range(NPAIR)]).astype(bf16)
        wv = np.concatenate([Wv[hs + h] for h in range(HPC)], axis=1).astype(bf16)
        wo = np.ascontiguousarray(Wo[hs * DK:(hs + HPC) * DK, :]).astype(bf16)
        in_maps.append({"xt": xt, "wq": wq, "wk": wk, "wv": wv, "wo": wo,
                        "ident": ident})

    nc = _get_nc()
    res = run_bass_kernel_spmd(nc, in_maps, core_ids=list(range(8))).results

    out = np.empty((B, S, D), dtype=np.float32)
    for b in range(B):
        out[b] = res[2 * b]["out"] + res[2 * b + 1]["out"]
    return out


# revision 7
# speedup vs baseline: 1.0675x; 1.0494x over previous
"""Multi-head attention (B=4, S=2048, D=1024, H=16, dk=dv=64) on 8 TRN2 cores.

Sharding: core = (batch b, head-group g): data-parallel over batch (4) x
tensor-parallel over heads (2 groups of 8). Each core computes its batch's
Q/K/V projections for its 8 heads, attention, and a partial output
projection over its heads' rows of Wo. The host sums the two partial
outputs per batch.

Per-core kernel, all matmul operands bf16 (fp32 PSUM accumulate):
  xT is DMA'd once into SBUF (bf16, 32KB/partition) and reused by all
  projections. Heads are processed in 4 pairs; per pair Q/K are projected
  pair-packed ([2x64 dk, S]).

  Attention is a flat pipeline over (pair, query-block, key-chunk) steps
  paced by the ACT engine (exp of the 128x1024 score tile is the global
  floor at ~266us/core). Per key chunk both heads' scores land in one
  [128, 1024] PSUM tile and one exp covers both. The AV matmul is
  query-stationary: lhsT = exp-tile slice [128 s, 128 q], rhs =
  [V_h | ones] [128 s, 65] -> PSUM [128 q, 65] accumulated over 16 key
  chunks; column 64 is the softmax denominator (65 rows/matmul instead of
  512 for the value-stationary form). Only the first of the four
  query-chunk groups per PSUM bank passes start=True: start zeroes the
  whole 2KB bank, so the other groups bootstrap off the bank-wide
  pending-zero. After a block's 16 chunks the accumulator is copied to
  SBUF (freeing the bank), normalized with a DVE reciprocal +
  per-partition-scalar multiply, and PE-transposed back to [hv, q] for
  the output projection.

  All projection / V / output-projection matmul groups are emitted at a
  strongly deprioritized Tile priority: the list scheduler then slots
  them into PE gaps behind the attention stream instead of ahead of it,
  which keeps the ACT queue fed.
"""

import numpy as np

import concourse.bacc as bacc
import concourse.tile as tile
import concourse.mybir as mybir
from concourse.bass_utils import run_bass_kernel_spmd

F32 = mybir.dt.float32
BF16 = mybir.dt.bfloat16
EXP = mybir.ActivationFunctionType.Exp

P = 128
S = 2048
D = 1024
DK = 64
HPC = 8            # heads per core
NSC = S // P       # 16 key chunks of 128
NST = 4            # s-tiles of 512
STW = S // NST     # 512
NDC = D // P       # 8 d_model chunks
NPAIR = HPC // 2   # 4 head pairs
NQB = S // STW     # 4 query blocks of 512
NQC = STW // P     # 4 query chunks of 128 per block
SCALE = 1.0 / np.sqrt(DK)
LOWP = 10**7       # priority offset pushing filler behind the attention stream


def build_kernel():
    nc = bacc.Bacc("TRN2", target_bir_lowering=False, debug=False)

    xt_d = nc.dram_tensor("xt", [D, S], BF16, kind="ExternalInput")
    wq_d = nc.dram_tensor("wq", [NPAIR, D, P], BF16, kind="ExternalInput")
    wk_d = nc.dram_tensor("wk", [NPAIR, D, P], BF16, kind="ExternalInput")
    wv_d = nc.dram_tensor("wv", [D, HPC * DK], BF16, kind="ExternalInput")
    wo_d = nc.dram_tensor("wo", [HPC * DK, D], BF16, kind="ExternalInput")
    id_d = nc.dram_tensor("ident", [P, P], BF16, kind="ExternalInput")
    out_d = nc.dram_tensor("out", [S, D], F32, kind="ExternalOutput")

    xt_ap = xt_d.ap().rearrange("(dc p) s -> p dc s", p=P)

    with tile.TileContext(nc) as tc:
        with tc.tile_pool(name="persist", bufs=1) as persist, \
             tc.tile_pool(name="qkpool", bufs=2) as qkpool, \
             tc.tile_pool(name="wqkp", bufs=2) as wqkp, \
             tc.tile_pool(name="expp", bufs=3) as expp, \
             tc.tile_pool(name="avsbp", bufs=2) as avsbp, \
             tc.tile_pool(name="htsp", bufs=3) as htsp, \
             tc.tile_pool(name="recp", bufs=4) as recp, \
             tc.tile_pool(name="stage", bufs=2) as stage, \
             tc.tile_pool(name="scps", bufs=2, space="PSUM") as scps, \
             tc.tile_pool(name="avps", bufs=1, space="PSUM") as avps, \
             tc.tile_pool(name="qkps", bufs=2, space="PSUM") as qkps:

            xts = persist.tile([P, NDC, S], BF16, tag="xts")
            v520 = persist.tile([P, NSC, HPC, DK + 1], BF16, tag="v520")
            hn = persist.tile([P, NPAIR, S], BF16, tag="hn")
            wo_sb = persist.tile([P, NPAIR, D], BF16, tag="wo")
            wv_sb = persist.tile([P, NDC, HPC * DK], BF16, tag="wv")
            id_sb = persist.tile([P, P], BF16, tag="id")

            # ---------------- prologue DMAs ----------------
            wqk_tiles = {}

            def fetch_wqk(pr):
                wqp = wqkp.tile([P, NDC, P], BF16, tag="wqp", name=f"wqp{pr}")
                wkp = wqkp.tile([P, NDC, P], BF16, tag="wkp", name=f"wkp{pr}")
                nc.sync.dma_start(
                    wqp[:], wq_d.ap()[pr].rearrange("(dc p) c -> p dc c", p=P))
                nc.sync.dma_start(
                    wkp[:], wk_d.ap()[pr].rearrange("(dc p) c -> p dc c", p=P))
                wqk_tiles[pr] = (wqp, wkp)

            fetch_wqk(0)
            # xT in 16 chunks; query-block-0 columns first so pair-0 can start
            for qtr in range(4):
                for dh in range(NDC // 2):
                    nc.sync.dma_start(
                        xts[:, 2 * dh:2 * dh + 2, qtr * 512:(qtr + 1) * 512],
                        xt_ap[:, 2 * dh:2 * dh + 2, qtr * 512:(qtr + 1) * 512])
            nc.sync.dma_start(
                wv_sb[:], wv_d.ap().rearrange("(dc p) c -> p dc c", p=P))
            nc.sync.dma_start(id_sb[:], id_d.ap())
            nc.sync.dma_start(
                wo_sb[:], wo_d.ap().rearrange("(ci p) d -> p ci d", p=P))
            nc.vector.memset(v520[:, :, :, DK:DK + 1], 1.0)

            qk_tiles = {0: (qkpool.tile([P, S], BF16, tag="qt", name="qt0"),
                            qkpool.tile([P, S], BF16, tag="kt", name="kt0"))}

            # ---------------- helpers ----------------
            def qk_group(pr, which, st, sub_sc=False):
                """Project qtp/ktp columns st*512:(st+1)*512 for pair pr."""
                w_sb = wqk_tiles[pr][0 if which == "q" else 1]
                dst = qk_tiles[pr][0 if which == "q" else 1]
                ps = qkps.tile([P, STW], F32, tag="qkp", name=f"{which}{pr}{st}")
                if sub_sc:
                    # key-chunk granular psum + copies (fast prologue start)
                    for scl in range(4):
                        for dc in range(NDC):
                            nc.tensor.matmul(
                                ps[:, scl * P:(scl + 1) * P], w_sb[:, dc, :],
                                xts[:, dc, (st * 4 + scl) * P:(st * 4 + scl + 1) * P],
                                start=(dc == 0), stop=(dc == NDC - 1))
                        nc.vector.tensor_copy(
                            dst[:, (st * 4 + scl) * P:(st * 4 + scl + 1) * P],
                            ps[:, scl * P:(scl + 1) * P])
                else:
                    for dc in range(NDC):
                        nc.tensor.matmul(
                            ps[:], w_sb[:, dc, :],
                            xts[:, dc, st * STW:(st + 1) * STW],
                            start=(dc == 0), stop=(dc == NDC - 1))
                    nc.vector.tensor_copy(
                        dst[:, st * STW:(st + 1) * STW], ps[:])

            def v_group(pr, sc):
                """Project V for pair pr's two heads, key chunk sc."""
                ps = qkps.tile([P, STW], F32, tag="qkp", name=f"v{pr}{sc}")
                for dc in range(NDC):
                    nc.tensor.matmul(
                        ps[:, 0:P], xts[:, dc, sc * P:(sc + 1) * P],
                        wv_sb[:, dc, pr * P:(pr + 1) * P],
                        start=(dc == 0), stop=(dc == NDC - 1))
                nc.vector.tensor_copy(
                    v520[:, sc, 2 * pr:2 * pr + 2, 0:DK],
                    ps[:, 0:P].rearrange("p (h v) -> p h v", v=DK))

            def out_group(sc_out, dmh):
                ps = qkps.tile([P, STW], F32, tag="qkp", name=f"o{sc_out}{dmh}")
                for ci in range(NPAIR):
                    nc.tensor.matmul(
                        ps[:], hn[:, ci, sc_out * P:(sc_out + 1) * P],
                        wo_sb[:, ci, dmh * 512:(dmh + 1) * 512],
                        start=(ci == 0), stop=(ci == NPAIR - 1))
                osb = stage.tile([P, STW], F32, tag="ostage", name="osb")
                nc.vector.tensor_copy(osb[:], ps[:])
                nc.sync.dma_start(
                    out_d.ap()[sc_out * P:(sc_out + 1) * P,
                               dmh * 512:(dmh + 1) * 512],
                    osb[:])

            def scores_exp(pr, qb, sc):
                qtp, ktp = qk_tiles[pr]
                q0 = qb * STW
                scp = scps.tile([P, 2 * STW], F32, tag="scp", name="scp")
                for j in range(2):
                    nc.tensor.matmul(
                        scp[:, j * STW:(j + 1) * STW],
                        ktp[j * DK:(j + 1) * DK, sc * P:(sc + 1) * P],
                        qtp[j * DK:(j + 1) * DK, q0:q0 + STW],
                        start=True, stop=True)
                ex = expp.tile([P, 2 * STW], BF16, tag="ex", name="ex")
                nc.scalar.activation(ex[:], scp[:], EXP, scale=float(SCALE))
                return ex

            av_cur = [None]

            def av_mms(pr, qb, sc, ex):
                if sc == 0:
                    av_cur[0] = [avps.tile([P, NQC * 65], F32, tag=f"av{j}",
                                           name=f"av{j}_{pr}{qb}")
                                 for j in range(2)]
                av = av_cur[0]
                for j in range(2):
                    for qc in range(NQC):
                        nc.tensor.matmul(
                            av[j][:, qc * 65:(qc + 1) * 65],
                            ex[:, j * STW + qc * P:j * STW + (qc + 1) * P],
                            v520[:, sc, 2 * pr + j, :],
                            # start=True zeroes the whole 2KB psum bank, so
                            # only the first of the 4 interleaved qc groups
                            # starts; the rest overwrite-on-first-write via
                            # the bank-wide pending-zero.
                            start=(sc == 0 and qc == 0),
                            stop=(sc == NSC - 1),
                            skip_group_check=True)
                if sc == NSC - 1:
                    avsb = avsbp.tile([P, 2, NQC * 65], F32, tag="avsb",
                                      name=f"avsb{pr}{qb}")
                    for j in range(2):
                        nc.vector.tensor_copy(avsb[:, j, :], av[j][:])
                    return avsb
                return None

            def norm_qc(pr, qb, avsb, qc):
                """Normalize + transpose one 128-query chunk of both heads."""
                hts = htsp.tile([P, P], BF16, tag="hts", name=f"h{pr}{qb}{qc}")
                for j in range(2):
                    rec = recp.tile([P, 1], F32, tag="rec", name="rec")
                    nc.vector.reciprocal(
                        rec[:], avsb[:, j, qc * 65 + DK:qc * 65 + DK + 1])
                    nc.vector.tensor_scalar_mul(
                        hts[:, j * DK:(j + 1) * DK],
                        avsb[:, j, qc * 65:qc * 65 + DK], rec[:])
                tp = qkps.tile([P, STW], F32, tag="qkp", name=f"t{pr}{qb}{qc}")
                tpb = tp[:, 0:P // 2].bitcast(BF16)
                nc.tensor.transpose(tpb, hts[:], id_sb[:])
                nc.vector.tensor_copy(
                    hn[:, pr, (qb * NQC + qc) * P:(qb * NQC + qc + 1) * P],
                    tpb)

            # ---------------- prologue compute (pair 0 heads) --------------
            qk_group(0, "q", 0, sub_sc=True)
            qk_group(0, "k", 0, sub_sc=True)
            for sc in range(4):
                v_group(0, sc)
            with tc.high_priority(offset=-LOWP):
                for st in range(1, NST):
                    qk_group(0, "q", st)
                    qk_group(0, "k", st)
                for sc in range(4, NSC):
                    v_group(0, sc)

            # ---------------- flat attention pipeline ----------------
            steps = [(pr, qb, sc)
                     for pr in range(NPAIR)
                     for qb in range(NQB)
                     for sc in range(NSC)]
            exs = {}
            for t in range(len(steps) + 1):
                if t < len(steps):
                    cur = steps[t]
                    if cur[1:] == (0, 0) and cur[0] + 1 < NPAIR:
                        # stage next pair's projections as low-priority filler
                        nxt = cur[0] + 1
                        qk_tiles[nxt] = (
                            qkpool.tile([P, S], BF16, tag="qt", name=f"qt{nxt}"),
                            qkpool.tile([P, S], BF16, tag="kt", name=f"kt{nxt}"))
                        fetch_wqk(nxt)
                        with tc.high_priority(offset=-LOWP):
                            for st in range(NST):
                                qk_group(nxt, "q", st)
                                qk_group(nxt, "k", st)
                            for sc in range(NSC):
                                v_group(nxt, sc)
                    exs[cur] = scores_exp(*cur)
                if t > 0:
                    prv = steps[t - 1]
                    avsb = av_mms(*prv, exs.pop(prv))
                    if avsb is not None:
                        ppr, pqb = prv[0], prv[1]
                        for qc in range(NQC):
                            norm_qc(ppr, pqb, avsb, qc)
                            if ppr == NPAIR - 1:
                                so = pqb * NQC + qc
                                if pqb < NQB - 1:
                                    with tc.high_priority(offset=-LOWP):
                                        out_group(so, 0)
                                        out_group(so, 1)
                                else:
                                    out_group(so, 0)
                                    out_group(so, 1)

    nc.compile()
    return nc


_NC_CACHE = None


def _get_nc():
    global _NC_CACHE
    if _NC_CACHE is None:
        _NC_CACHE = build_kernel()
    return _NC_CACHE


def kernel(x, Wq, Wk, Wv, Wo):
    import ml_dtypes
    bf16 = ml_dtypes.bfloat16

    x = np.asarray(x, dtype=np.float32)
    Wq = np.asarray(Wq, dtype=np.float32)
    Wk = np.asarray(Wk, dtype=np.float32)
    Wv = np.asarray(Wv, dtype=np.float32)
    Wo = np.asarray(Wo, dtype=np.float32)
    B = x.shape[0]
    ident = np.eye(P, dtype=bf16)

    in_maps = []
    for core in range(8):
        b, g = divmod(core, 2)
        hs = g * HPC
        xt = np.ascontiguousarray(x[b].T).astype(bf16)
        wq = np.stack([
            np.concatenate([Wq[hs + 2 * p], Wq[hs + 2 * p + 1]], axis=1)
            for p in range(NPAIR)]).astype(bf16)
        wk = np.stack([
            np.concatenate([Wk[hs + 2 * p], Wk[hs + 2 * p + 1]], axis=1)
            for p in range(NPAIR)]).astype(bf16)
        wv = np.concatenate([Wv[hs + h] for h in range(HPC)], axis=1).astype(bf16)
        wo = np.ascontiguousarray(Wo[hs * DK:(hs + HPC) * DK, :]).astype(bf16)
        in_maps.append({"xt": xt, "wq": wq, "wk": wk, "wv": wv, "wo": wo,
                        "ident": ident})

    nc = _get_nc()
    res = run_bass_kernel_spmd(nc, in_maps, core_ids=list(range(8))).results

    out = np.empty((B, S, D), dtype=np.float32)
    for b in range(B):
        out[b] = res[2 * b]["out"] + res[2 * b + 1]["out"]
    return out


# revision 12
# speedup vs baseline: 1.1039x; 1.0341x over previous
"""Multi-head attention (B=4, S=2048, D=1024, H=16, dk=dv=64) on 8 TRN2 cores.

Sharding: core = (batch b, head-group g): data-parallel over batch (4) x
tensor-parallel over heads (2 groups of 8). Each core computes its batch's
Q/K/V projections for its 8 heads, attention, and a partial output
projection over its heads' rows of Wo. The host sums the two partial
outputs per batch.

Per-core kernel, all matmul operands bf16 (fp32 PSUM accumulate):
  xT is DMA'd once into SBUF (bf16, 32KB/partition) and reused by all
  projections. Heads are processed in 4 pairs; per pair Q/K are projected
  pair-packed ([2x64 dk, S]).

  Attention is a flat pipeline over (pair, query-block, key-chunk) steps
  paced by the ACT engine (exp of the 128x1024 score tile is the global
  floor at ~266us/core). Per key chunk both heads' scores land in one
  [128, 1024] PSUM tile and one exp covers both. The AV matmul is
  query-stationary: lhsT = exp-tile slice [128 s, 128 q], rhs =
  [V_h | ones] [128 s, 65] -> PSUM [128 q, 65] accumulated over 16 key
  chunks; column 64 is the softmax denominator (65 rows/matmul instead of
  512 for the value-stationary form). Only the first of the four
  query-chunk groups per PSUM bank passes start=True: start zeroes the
  whole 2KB bank, so the other groups bootstrap off the bank-wide
  pending-zero. After a block's 16 chunks the accumulator is copied to
  SBUF (freeing the bank), normalized with a DVE reciprocal +
  per-partition-scalar multiply, and PE-transposed back to [hv, q] for
  the output projection.

  All projection / V / output-projection matmul groups are emitted at a
  strongly deprioritized Tile priority: the list scheduler then slots
  them into PE gaps behind the attention stream instead of ahead of it,
  which keeps the ACT queue fed.
"""

import numpy as np

import concourse.bacc as bacc
import concourse.tile as tile
import concourse.mybir as mybir
from concourse.bass_utils import run_bass_kernel_spmd

F32 = mybir.dt.float32
BF16 = mybir.dt.bfloat16
EXP = mybir.ActivationFunctionType.Exp

P = 128
S = 2048
D = 1024
DK = 64
HPC = 8            # heads per core
NSC = S // P       # 16 key chunks of 128
NST = 4            # s-tiles of 512
STW = S // NST     # 512
NDC = D // P       # 8 d_model chunks
NPAIR = HPC // 2   # 4 head pairs
NQB = S // STW     # 4 query blocks of 512
NQC = STW // P     # 4 query chunks of 128 per block
SCALE = 1.0 / np.sqrt(DK)
LOWP = 10**7       # priority offset pushing filler behind the attention stream


def build_kernel():
    nc = bacc.Bacc("TRN2", target_bir_lowering=False, debug=False)

    xt_d = nc.dram_tensor("xt", [D, S], BF16, kind="ExternalInput")
    wq_d = nc.dram_tensor("wq", [NPAIR, D, P], BF16, kind="ExternalInput")
    wk_d = nc.dram_tensor("wk", [NPAIR, D, P], BF16, kind="ExternalInput")
    wv_d = nc.dram_tensor("wv", [D, HPC * DK], BF16, kind="ExternalInput")
    wo_d = nc.dram_tensor("wo", [HPC * DK, D], BF16, kind="ExternalInput")
    id_d = nc.dram_tensor("ident", [P, P], BF16, kind="ExternalInput")
    out_d = nc.dram_tensor("out", [S, D], F32, kind="ExternalOutput")

    xt_ap = xt_d.ap().rearrange("(dc p) s -> p dc s", p=P)

    with tile.TileContext(nc) as tc:
        with tc.tile_pool(name="persist", bufs=1) as persist, \
             tc.tile_pool(name="qkpool", bufs=2) as qkpool, \
             tc.tile_pool(name="wqkp", bufs=2) as wqkp, \
             tc.tile_pool(name="expp", bufs=3) as expp, \
             tc.tile_pool(name="avsbp", bufs=2) as avsbp, \
             tc.tile_pool(name="htsp", bufs=3) as htsp, \
             tc.tile_pool(name="recp", bufs=4) as recp, \
             tc.tile_pool(name="stage", bufs=2) as stage, \
             tc.tile_pool(name="scps", bufs=2, space="PSUM") as scps, \
             tc.tile_pool(name="avps", bufs=1, space="PSUM") as avps, \
             tc.tile_pool(name="qkps", bufs=2, space="PSUM") as qkps:

            xts = persist.tile([P, NDC, S], BF16, tag="xts")
            v520 = persist.tile([P, NSC, HPC, DK + 1], BF16, tag="v520")
            hn = persist.tile([P, NPAIR, S], BF16, tag="hn")
            wo_sb = persist.tile([P, NPAIR, D], BF16, tag="wo")
            wv_sb = persist.tile([P, NDC, HPC * DK], BF16, tag="wv")
            id_sb = persist.tile([P, P], BF16, tag="id")

            # ---------------- prologue DMAs ----------------
            wqk_tiles = {}

            def fetch_wqk(pr):
                wqp = wqkp.tile([P, NDC, P], BF16, tag="wqp", name=f"wqp{pr}")
                wkp = wqkp.tile([P, NDC, P], BF16, tag="wkp", name=f"wkp{pr}")
                nc.sync.dma_start(
                    wqp[:], wq_d.ap()[pr].rearrange("(dc p) c -> p dc c", p=P))
                nc.sync.dma_start(
                    wkp[:], wk_d.ap()[pr].rearrange("(dc p) c -> p dc c", p=P))
                wqk_tiles[pr] = (wqp, wkp)

            fetch_wqk(0)
            # xT in 16 chunks; query-block-0 columns first so pair-0 can
            # start; wv before the later chunks (V projection rides early),
            # ident/wo last (needed only from the first norm / pair 3).
            def xts_qtr(qtr):
                for dh in range(NDC // 2):
                    nc.sync.dma_start(
                        xts[:, 2 * dh:2 * dh + 2, qtr * 512:(qtr + 1) * 512],
                        xt_ap[:, 2 * dh:2 * dh + 2, qtr * 512:(qtr + 1) * 512])

            xts_qtr(0)
            nc.sync.dma_start(
                wv_sb[:], wv_d.ap().rearrange("(dc p) c -> p dc c", p=P))
            for qtr in range(1, 4):
                xts_qtr(qtr)
            nc.sync.dma_start(id_sb[:], id_d.ap())
            nc.sync.dma_start(
                wo_sb[:], wo_d.ap().rearrange("(ci p) d -> p ci d", p=P))
            nc.vector.memset(v520[:, :, :, DK:DK + 1], 1.0)

            qk_tiles = {0: (qkpool.tile([P, S], BF16, tag="qt", name="qt0"),
                            qkpool.tile([P, S], BF16, tag="kt", name="kt0"))}

            # ---------------- helpers ----------------
            def qk_group(pr, which, st, sub_sc=False):
                """Project qtp/ktp columns st*512:(st+1)*512 for pair pr."""
                w_sb = wqk_tiles[pr][0 if which == "q" else 1]
                dst = qk_tiles[pr][0 if which == "q" else 1]
                ps = qkps.tile([P, STW], F32, tag="qkp", name=f"{which}{pr}{st}")
                if sub_sc:
                    # key-chunk granular psum + copies (fast prologue start)
                    for scl in range(4):
                        for dc in range(NDC):
                            nc.tensor.matmul(
                                ps[:, scl * P:(scl + 1) * P], w_sb[:, dc, :],
                                xts[:, dc, (st * 4 + scl) * P:(st * 4 + scl + 1) * P],
                                start=(dc == 0), stop=(dc == NDC - 1))
                        nc.vector.tensor_copy(
                            dst[:, (st * 4 + scl) * P:(st * 4 + scl + 1) * P],
                            ps[:, scl * P:(scl + 1) * P])
                else:
                    for dc in range(NDC):
                        nc.tensor.matmul(
                            ps[:], w_sb[:, dc, :],
                            xts[:, dc, st * STW:(st + 1) * STW],
                            start=(dc == 0), stop=(dc == NDC - 1))
                    nc.vector.tensor_copy(
                        dst[:, st * STW:(st + 1) * STW], ps[:])

            def v_group(pr, sc):
                """Project V for pair pr's two heads, key chunk sc."""
                ps = qkps.tile([P, STW], F32, tag="qkp", name=f"v{pr}{sc}")
                for dc in range(NDC):
                    nc.tensor.matmul(
                        ps[:, 0:P], xts[:, dc, sc * P:(sc + 1) * P],
                        wv_sb[:, dc, pr * P:(pr + 1) * P],
                        start=(dc == 0), stop=(dc == NDC - 1))
                nc.vector.tensor_copy(
                    v520[:, sc, 2 * pr:2 * pr + 2, 0:DK],
                    ps[:, 0:P].rearrange("p (h v) -> p h v", v=DK))

            def out_group(sc_out, dmh, pool=None):
                if pool is scps:
                    pa = pool.tile([P, 2 * STW], F32, tag="scp",
                                   name=f"o{sc_out}{dmh}")[:, 0:STW]
                else:
                    pa = qkps.tile([P, STW], F32, tag="qkp",
                                   name=f"o{sc_out}{dmh}")[:]
                for ci in range(NPAIR):
                    nc.tensor.matmul(
                        pa, hn[:, ci, sc_out * P:(sc_out + 1) * P],
                        wo_sb[:, ci, dmh * 512:(dmh + 1) * 512],
                        start=(ci == 0), stop=(ci == NPAIR - 1))
                osb = stage.tile([P, STW], F32, tag="ostage", name="osb")
                nc.vector.tensor_copy(osb[:], pa)
                nc.sync.dma_start(
                    out_d.ap()[sc_out * P:(sc_out + 1) * P,
                               dmh * 512:(dmh + 1) * 512],
                    osb[:])

            def scores_exp(pr, qb, sc):
                qtp, ktp = qk_tiles[pr]
                q0 = qb * STW
                scp = scps.tile([P, 2 * STW], F32, tag="scp", name="scp")
                for j in range(2):
                    nc.tensor.matmul(
                        scp[:, j * STW:(j + 1) * STW],
                        ktp[j * DK:(j + 1) * DK, sc * P:(sc + 1) * P],
                        qtp[j * DK:(j + 1) * DK, q0:q0 + STW],
                        start=True, stop=True)
                ex = expp.tile([P, 2 * STW], BF16, tag="ex", name="ex")
                nc.scalar.activation(ex[:], scp[:], EXP, scale=float(SCALE))
                return ex

            av_cur = [None]

            def av_mms(pr, qb, sc, ex):
                if sc == 0:
                    av_cur[0] = [avps.tile([P, NQC * 65], F32, tag=f"av{j}",
                                           name=f"av{j}_{pr}{qb}")
                                 for j in range(2)]
                av = av_cur[0]
                for j in range(2):
                    for qc in range(NQC):
                        nc.tensor.matmul(
                            av[j][:, qc * 65:(qc + 1) * 65],
                            ex[:, j * STW + qc * P:j * STW + (qc + 1) * P],
                            v520[:, sc, 2 * pr + j, :],
                            # start=True zeroes the whole 2KB psum bank, so
                            # only the first of the 4 interleaved qc groups
                            # starts; the rest overwrite-on-first-write via
                            # the bank-wide pending-zero.
                            start=(sc == 0 and qc == 0),
                            stop=(sc == NSC - 1),
                            skip_group_check=True)
                if sc == NSC - 1:
                    avsb = avsbp.tile([P, 2, NQC * 65], F32, tag="avsb",
                                      name=f"avsb{pr}{qb}")
                    for j in range(2):
                        nc.vector.tensor_copy(avsb[:, j, :], av[j][:])
                    return avsb
                return None

            def norm_qc(pr, qb, avsb, qc):
                """Normalize + transpose one 128-query chunk of both heads.

                The transpose lands in the (between-blocks idle) av0 PSUM
                bank rather than the qkp pool, keeping the qkp rotation
                free for projection / output-projection groups. Emission
                order guarantees the next block's AV matmuls (which re-start
                the bank) come after the hn copies below.
                """
                hts = htsp.tile([P, P], BF16, tag="hts", name=f"h{pr}{qb}{qc}")
                for j in range(2):
                    rec = recp.tile([P, 1], F32, tag="rec", name="rec")
                    nc.vector.reciprocal(
                        rec[:], avsb[:, j, qc * 65 + DK:qc * 65 + DK + 1])
                    nc.vector.tensor_scalar_mul(
                        hts[:, j * DK:(j + 1) * DK],
                        avsb[:, j, qc * 65:qc * 65 + DK], rec[:])
                tpb = av_cur[0][0][:, qc * DK:(qc + 1) * DK].bitcast(BF16)
                nc.tensor.transpose(tpb, hts[:], id_sb[:])
                nc.vector.tensor_copy(
                    hn[:, pr, (qb * NQC + qc) * P:(qb * NQC + qc + 1) * P],
                    tpb)

            # ---------------- prologue compute (pair 0 heads) --------------
            qk_group(0, "q", 0, sub_sc=True)
            qk_group(0, "k", 0, sub_sc=True)
            for sc in range(4):
                v_group(0, sc)
            with tc.high_priority(offset=-LOWP):
                for st in range(1, NST):
                    qk_group(0, "q", st)
                    qk_group(0, "k", st)
                for sc in range(4, NSC):
                    v_group(0, sc)

            # ---------------- flat attention pipeline ----------------
            steps = [(pr, qb, sc)
                     for pr in range(NPAIR)
                     for qb in range(NQB)
                     for sc in range(NSC)]
            exs = {}
            for t in range(len(steps) + 1):
                if t < len(steps):
                    cur = steps[t]
                    if cur[1:] == (0, 0) and cur[0] + 1 < NPAIR:
                        # stage next pair's projections as low-priority filler
                        nxt = cur[0] + 1
                        qk_tiles[nxt] = (
                            qkpool.tile([P, S], BF16, tag="qt", name=f"qt{nxt}"),
                            qkpool.tile([P, S], BF16, tag="kt", name=f"kt{nxt}"))
                        fetch_wqk(nxt)
                        with tc.high_priority(offset=-LOWP):
                            for st in range(NST):
                                qk_group(nxt, "q", st)
                                qk_group(nxt, "k", st)
                            for sc in range(NSC):
                                v_group(nxt, sc)
                    exs[cur] = scores_exp(*cur)
                if t > 0:
                    prv = steps[t - 1]
                    avsb = av_mms(*prv, exs.pop(prv))
                    if avsb is not None:
                        ppr, pqb = prv[0], prv[1]
                        for qc in range(NQC):
                            norm_qc(ppr, pqb, avsb, qc)
                            if ppr == NPAIR - 1:
                                so = pqb * NQC + qc
                                if pqb < NQB - 1:
                                    with tc.high_priority(offset=-LOWP):
                                        out_group(so, 0)
                                        out_group(so, 1)
                                else:
                                    # epilogue: scores are done, so borrow
                                    # the scp banks for a 4-deep rotation
                                    out_group(so, 0, pool=scps)
                                    out_group(so, 1)

    nc.compile()
    return nc


_NC_CACHE = None


def _get_nc():
    global _NC_CACHE
    if _NC_CACHE is None:
        _NC_CACHE = build_kernel()
    return _NC_CACHE


def kernel(x, Wq, Wk, Wv, Wo):
    import ml_dtypes
    bf16 = ml_dtypes.bfloat16

    x = np.asarray(x, dtype=np.float32)
    Wq = np.asarray(Wq, dtype=np.float32)
    Wk = np.asarray(Wk, dtype=np.float32)
    Wv = np.asarray(Wv, dtype=np.float32)
    Wo = np.asarray(Wo, dtype=np.float32)
    B = x.shape[0]
    ident = np.eye(P, dtype=bf16)

    in_maps = []
    for core in range(8):
        b, g = divmod(core, 2)
        hs = g * HPC
        xt = np.ascontiguousarray(x[b].T).astype(bf16)
        wq = np.stack([
            np.concatenate([Wq[hs + 2 * p], Wq[hs + 2 * p + 1]], axis=1)
            for p in range(NPAIR)]).astype(bf16)
        wk = np.stack([
            np.concatenate([Wk[hs + 2 * p], Wk[hs + 2 * p + 1]], axis=1)
            for p in range(NPAIR)]).astype(bf16)
        wv = np.concatenate([Wv[hs + h] for h in range(HPC)], axis=1).astype(bf16)
        wo = np.ascontiguousarray(Wo[hs * DK:(hs + HPC) * DK, :]).astype(bf16)
        in_maps.append({"xt": xt, "wq": wq, "wk": wk, "wv": wv, "wo": wo,
                        "ident": ident})

    nc = _get_nc()
    res = run_bass_kernel_spmd(nc, in_maps, core_ids=list(range(8))).results

    out = np.empty((B, S, D), dtype=np.float32)
    for b in range(B):
        out[b] = res[2 * b]["out"] + res[2 * b + 1]["out"]
    return out


# revision 16
# speedup vs baseline: 1.1552x; 1.0465x over previous
"""Multi-head attention (B=4, S=2048, D=1024, H=16, dk=dv=64) on 8 TRN2 cores.

Sharding: core = (batch b, head-group g): data-parallel over batch (4) x
tensor-parallel over heads (2 groups of 8). Each core computes its batch's
Q/K/V projections for its 8 heads, attention, and a partial output
projection over its heads' rows of Wo. The host sums the two partial
outputs per batch.

Per-core kernel, all matmul operands bf16 (fp32 PSUM accumulate):
  xT is DMA'd once into SBUF (bf16, 32KB/partition) and reused by all
  projections. Heads are processed in 4 pairs; per pair Q/K are projected
  pair-packed ([2x64 dk, S]).

  Attention is a flat pipeline over (pair, query-block, key-chunk) steps
  paced by the ACT engine (exp of the 128x1024 score tile is the global
  floor at ~266us/core). Per key chunk both heads' scores land in one
  [128, 1024] PSUM tile and one exp covers both. The AV matmul is
  query-stationary: lhsT = exp-tile slice [128 s, 128 q], rhs =
  [V_h | ones] [128 s, 65] -> PSUM [128 q, 65] accumulated over 16 key
  chunks; column 64 is the softmax denominator (65 rows/matmul instead of
  512 for the value-stationary form). Only the first of the four
  query-chunk groups per PSUM bank passes start=True: start zeroes the
  whole 2KB bank, so the other groups bootstrap off the bank-wide
  pending-zero. After a block's 16 chunks the accumulator is copied to
  SBUF (freeing the bank), normalized with a DVE reciprocal +
  per-partition-scalar multiply, and PE-transposed back to [hv, q] for
  the output projection.

  All projection / V / output-projection matmul groups are emitted at a
  strongly deprioritized Tile priority: the list scheduler then slots
  them into PE gaps behind the attention stream instead of ahead of it,
  which keeps the ACT queue fed.
"""

import numpy as np

import concourse.bacc as bacc
import concourse.tile as tile
import concourse.mybir as mybir
from concourse.bass_utils import run_bass_kernel_spmd

F32 = mybir.dt.float32
BF16 = mybir.dt.bfloat16
EXP = mybir.ActivationFunctionType.Exp

P = 128
S = 2048
D = 1024
DK = 64
HPC = 8            # heads per core
NSC = S // P       # 16 key chunks of 128
NST = 4            # s-tiles of 512
STW = S // NST     # 512
NDC = D // P       # 8 d_model chunks
NPAIR = HPC // 2   # 4 head pairs
NQB = S // STW     # 4 query blocks of 512
NQC = STW // P     # 4 query chunks of 128 per block
SCALE = 1.0 / np.sqrt(DK)
LOWP = 10**7       # priority offset pushing filler behind the attention stream


def build_kernel():
    nc = bacc.Bacc("TRN2", target_bir_lowering=False, debug=False)

    xt_d = nc.dram_tensor("xt", [D, S], BF16, kind="ExternalInput")
    wq_d = nc.dram_tensor("wq", [NPAIR, D, P], BF16, kind="ExternalInput")
    wk_d = nc.dram_tensor("wk", [NPAIR, D, P], BF16, kind="ExternalInput")
    wv_d = nc.dram_tensor("wv", [D, HPC * DK], BF16, kind="ExternalInput")
    wo_d = nc.dram_tensor("wo", [HPC * DK, D], BF16, kind="ExternalInput")
    id_d = nc.dram_tensor("ident", [P, P], BF16, kind="ExternalInput")
    out_d = nc.dram_tensor("out", [S, D], F32, kind="ExternalOutput")

    xt_ap = xt_d.ap().rearrange("(dc p) s -> p dc s", p=P)

    with tile.TileContext(nc) as tc:
        with tc.tile_pool(name="persist", bufs=1) as persist, \
             tc.tile_pool(name="qkpool", bufs=2) as qkpool, \
             tc.tile_pool(name="wqkp", bufs=2) as wqkp, \
             tc.tile_pool(name="expp", bufs=6) as expp, \
             tc.tile_pool(name="avsbp", bufs=2) as avsbp, \
             tc.tile_pool(name="htsp", bufs=3) as htsp, \
             tc.tile_pool(name="recp", bufs=4) as recp, \
             tc.tile_pool(name="stage", bufs=2) as stage, \
             tc.tile_pool(name="scps", bufs=2, space="PSUM") as scps, \
             tc.tile_pool(name="avps", bufs=1, space="PSUM") as avps, \
             tc.tile_pool(name="qkps", bufs=2, space="PSUM") as qkps:

            xts = persist.tile([P, NDC, S], BF16, tag="xts")
            v520 = persist.tile([P, NSC, HPC, DK + 1], BF16, tag="v520")
            hn = persist.tile([P, NPAIR, S], BF16, tag="hn")
            wo_sb = persist.tile([P, NPAIR, D], BF16, tag="wo")
            wv_sb = persist.tile([P, NDC, HPC * DK], BF16, tag="wv")
            id_sb = persist.tile([P, P], BF16, tag="id")

            # ---------------- prologue DMAs ----------------
            wqk_tiles = {}

            def fetch_wqk(pr):
                wqp = wqkp.tile([P, NDC, P], BF16, tag="wqp", name=f"wqp{pr}")
                wkp = wqkp.tile([P, NDC, P], BF16, tag="wkp", name=f"wkp{pr}")
                nc.sync.dma_start(
                    wqp[:], wq_d.ap()[pr].rearrange("(dc p) c -> p dc c", p=P))
                nc.sync.dma_start(
                    wkp[:], wk_d.ap()[pr].rearrange("(dc p) c -> p dc c", p=P))
                wqk_tiles[pr] = (wqp, wkp)

            fetch_wqk(0)
            # xT in 16 chunks; query-block-0 columns first so pair-0 can
            # start; wv before the later chunks (V projection rides early),
            # ident/wo last (needed only from the first norm / pair 3).
            def xts_qtr(qtr):
                for dh in range(NDC // 2):
                    nc.sync.dma_start(
                        xts[:, 2 * dh:2 * dh + 2, qtr * 512:(qtr + 1) * 512],
                        xt_ap[:, 2 * dh:2 * dh + 2, qtr * 512:(qtr + 1) * 512])

            xts_qtr(0)
            xts_qtr(1)
            nc.sync.dma_start(
                wv_sb[:], wv_d.ap().rearrange("(dc p) c -> p dc c", p=P))
            for qtr in range(2, 4):
                xts_qtr(qtr)
            nc.sync.dma_start(id_sb[:], id_d.ap())
            nc.sync.dma_start(
                wo_sb[:], wo_d.ap().rearrange("(ci p) d -> p ci d", p=P))
            nc.vector.memset(v520[:, :, :, DK:DK + 1], 1.0)

            qk_tiles = {0: (qkpool.tile([P, S], BF16, tag="qt", name="qt0"),
                            qkpool.tile([P, S], BF16, tag="kt", name="kt0"))}

            # ---------------- helpers ----------------
            def qk_group(pr, which, st, sub_sc=False):
                """Project qtp/ktp columns st*512:(st+1)*512 for pair pr."""
                w_sb = wqk_tiles[pr][0 if which == "q" else 1]
                dst = qk_tiles[pr][0 if which == "q" else 1]
                ps = qkps.tile([P, STW], F32, tag="qkp", name=f"{which}{pr}{st}")
                if sub_sc:
                    # key-chunk granular psum + copies (fast prologue start)
                    for scl in range(4):
                        for dc in range(NDC):
                            nc.tensor.matmul(
                                ps[:, scl * P:(scl + 1) * P], w_sb[:, dc, :],
                                xts[:, dc, (st * 4 + scl) * P:(st * 4 + scl + 1) * P],
                                start=(dc == 0), stop=(dc == NDC - 1))
                        nc.vector.tensor_copy(
                            dst[:, (st * 4 + scl) * P:(st * 4 + scl + 1) * P],
                            ps[:, scl * P:(scl + 1) * P])
                else:
                    for dc in range(NDC):
                        nc.tensor.matmul(
                            ps[:], w_sb[:, dc, :],
                            xts[:, dc, st * STW:(st + 1) * STW],
                            start=(dc == 0), stop=(dc == NDC - 1))
                    nc.vector.tensor_copy(
                        dst[:, st * STW:(st + 1) * STW], ps[:])

            def v_group(pr, sc):
                """Project V for pair pr's two heads, key chunk sc."""
                ps = qkps.tile([P, STW], F32, tag="qkp", name=f"v{pr}{sc}")
                for dc in range(NDC):
                    nc.tensor.matmul(
                        ps[:, 0:P], xts[:, dc, sc * P:(sc + 1) * P],
                        wv_sb[:, dc, pr * P:(pr + 1) * P],
                        start=(dc == 0), stop=(dc == NDC - 1))
                nc.vector.tensor_copy(
                    v520[:, sc, 2 * pr:2 * pr + 2, 0:DK],
                    ps[:, 0:P].rearrange("p (h v) -> p h v", v=DK))

            def out_group(sc_out, dmh, pool=None):
                if pool is scps:
                    pa = pool.tile([P, 2 * STW], F32, tag="scp",
                                   name=f"o{sc_out}{dmh}")[:, 0:STW]
                else:
                    pa = qkps.tile([P, STW], F32, tag="qkp",
                                   name=f"o{sc_out}{dmh}")[:]
                for ci in range(NPAIR):
                    nc.tensor.matmul(
                        pa, hn[:, ci, sc_out * P:(sc_out + 1) * P],
                        wo_sb[:, ci, dmh * 512:(dmh + 1) * 512],
                        start=(ci == 0), stop=(ci == NPAIR - 1))
                osb = stage.tile([P, STW], F32, tag="ostage", name="osb")
                nc.vector.tensor_copy(osb[:], pa)
                nc.sync.dma_start(
                    out_d.ap()[sc_out * P:(sc_out + 1) * P,
                               dmh * 512:(dmh + 1) * 512],
                    osb[:])

            def scores_exp(pr, qb, sc):
                qtp, ktp = qk_tiles[pr]
                q0 = qb * STW
                scp = scps.tile([P, 2 * STW], F32, tag="scp", name="scp")
                for j in range(2):
                    nc.tensor.matmul(
                        scp[:, j * STW:(j + 1) * STW],
                        ktp[j * DK:(j + 1) * DK, sc * P:(sc + 1) * P],
                        qtp[j * DK:(j + 1) * DK, q0:q0 + STW],
                        start=True, stop=True)
                ex = expp.tile([P, 2 * STW], BF16, tag="ex", name="ex")
                nc.scalar.activation(ex[:], scp[:], EXP, scale=float(SCALE))
                return ex

            av_cur = [None]

            def av_mms(pr, qb, sc, ex):
                if sc == 0:
                    av_cur[0] = [avps.tile([P, NQC * 65], F32, tag=f"av{j}",
                                           name=f"av{j}_{pr}{qb}")
                                 for j in range(2)]
                av = av_cur[0]
                for j in range(2):
                    for qc in range(NQC):
                        nc.tensor.matmul(
                            av[j][:, qc * 65:(qc + 1) * 65],
                            ex[:, j * STW + qc * P:j * STW + (qc + 1) * P],
                            v520[:, sc, 2 * pr + j, :],
                            # start=True zeroes the whole 2KB psum bank, so
                            # only the first of the 4 interleaved qc groups
                            # starts; the rest overwrite-on-first-write via
                            # the bank-wide pending-zero.
                            start=(sc == 0 and qc == 0),
                            stop=(sc == NSC - 1),
                            skip_group_check=True)
                if sc == NSC - 1:
                    avsb = avsbp.tile([P, 2, NQC * 65], F32, tag="avsb",
                                      name=f"avsb{pr}{qb}")
                    for j in range(2):
                        nc.vector.tensor_copy(avsb[:, j, :], av[j][:])
                    return avsb
                return None

            def norm_qc(pr, qb, avsb, qc):
                """Normalize + transpose one 128-query chunk of both heads.

                The transpose lands in the (between-blocks idle) av0 PSUM
                bank rather than the qkp pool, keeping the qkp rotation
                free for projection / output-projection groups. Emission
                order guarantees the next block's AV matmuls (which re-start
                the bank) come after the hn copies below.
                """
                hts = htsp.tile([P, P], BF16, tag="hts", name=f"h{pr}{qb}{qc}")
                for j in range(2):
                    rec = recp.tile([P, 1], F32, tag="rec", name="rec")
                    nc.vector.reciprocal(
                        rec[:], avsb[:, j, qc * 65 + DK:qc * 65 + DK + 1])
                    nc.vector.tensor_scalar_mul(
                        hts[:, j * DK:(j + 1) * DK],
                        avsb[:, j, qc * 65:qc * 65 + DK], rec[:])
                tpb = av_cur[0][0][:, qc * DK:(qc + 1) * DK].bitcast(BF16)
                nc.tensor.transpose(tpb, hts[:], id_sb[:])
                nc.vector.tensor_copy(
                    hn[:, pr, (qb * NQC + qc) * P:(qb * NQC + qc + 1) * P],
                    tpb)

            # ---------------- prologue compute (pair 0 heads) --------------
            qk_group(0, "q", 0, sub_sc=True)
            qk_group(0, "k", 0, sub_sc=True)
            for sc in range(4):
                v_group(0, sc)
            with tc.high_priority(offset=-LOWP):
                for st in range(1, NST):
                    qk_group(0, "q", st)
                    qk_group(0, "k", st)
                for sc in range(4, NSC):
                    v_group(0, sc)

            # ---------------- flat attention pipeline ----------------
            steps = [(pr, qb, sc)
                     for pr in range(NPAIR)
                     for qb in range(NQB)
                     for sc in range(NSC)]
            exs = {}
            for t in range(len(steps) + 1):
                if t < len(steps):
                    cur = steps[t]
                    if cur[1:] == (0, 0):
                        pr0 = cur[0]
                        with tc.high_priority(offset=-LOWP):
                            if pr0 >= 1:
                                # this pair's late K columns / V chunks, only
                                # needed 8+ steps in: keep them in this
                                # pair's own span to unload the previous one
                                for st in range(2, NST):
                                    qk_group(pr0, "k", st)
                                for sc in range(NSC // 2, NSC):
                                    v_group(pr0, sc)
                        if pr0 + 1 < NPAIR:
                            # stage next pair's early projections as filler
                            nxt = pr0 + 1
                            qk_tiles[nxt] = (
                                qkpool.tile([P, S], BF16, tag="qt",
                                            name=f"qt{nxt}"),
                                qkpool.tile([P, S], BF16, tag="kt",
                                            name=f"kt{nxt}"))
                            fetch_wqk(nxt)
                            with tc.high_priority(offset=-LOWP):
                                for st in range(NST):
                                    qk_group(nxt, "q", st)
                                for st in range(2):
                                    qk_group(nxt, "k", st)
                                for sc in range(NSC // 2):
                                    v_group(nxt, sc)
                    exs[cur] = scores_exp(*cur)
                if t > 0:
                    prv = steps[t - 1]
                    avsb = av_mms(*prv, exs.pop(prv))
                    if avsb is not None:
                        ppr, pqb = prv[0], prv[1]
                        for qc in range(NQC):
                            norm_qc(ppr, pqb, avsb, qc)
                            if ppr == NPAIR - 1:
                                so = pqb * NQC + qc
                                if pqb < NQB - 1:
                                    with tc.high_priority(offset=-LOWP):
                                        out_group(so, 0)
                                        out_group(so, 1)
                                else:
                                    # epilogue: scores are done, so borrow
                                    # the scp banks for a 4-deep rotation
                                    out_group(so, 0, pool=scps)
                                    out_group(so, 1)

    nc.compile()
    return nc


_NC_CACHE = None


def _get_nc():
    global _NC_CACHE
    if _NC_CACHE is None:
        _NC_CACHE = build_kernel()
    return _NC_CACHE


def kernel(x, Wq, Wk, Wv, Wo):
    import ml_dtypes
    bf16 = ml_dtypes.bfloat16

    x = np.asarray(x, dtype=np.float32)
    Wq = np.asarray(Wq, dtype=np.float32)
    Wk = np.asarray(Wk, dtype=np.float32)
    Wv = np.asarray(Wv, dtype=np.float32)
    Wo = np.asarray(Wo, dtype=np.float32)
    B = x.shape[0]
    ident = np.eye(P, dtype=bf16)

    in_maps = []
    for core in range(8):
        b, g = divmod(core, 2)
        hs = g * HPC
        xt = np.ascontiguousarray(x[b].T).astype(bf16)
        wq = np.stack([
            np.concatenate([Wq[hs + 2 * p], Wq[hs + 2 * p + 1]], axis=1)
            for p in range(NPAIR)]).astype(bf16)
        wk = np.stack([
            np.concatenate([Wk[hs + 2 * p], Wk[hs + 2 * p + 1]], axis=1)
            for p in range(NPAIR)]).astype(bf16)
        wv = np.concatenate([Wv[hs + h] for h in range(HPC)], axis=1).astype(bf16)
        wo = np.ascontiguousarray(Wo[hs * DK:(hs + HPC) * DK, :]).astype(bf16)
        in_maps.append({"xt": xt, "wq": wq, "wk": wk, "wv": wv, "wo": wo,
                        "ident": ident})

    nc = _get_nc()
    res = run_bass_kernel_spmd(nc, in_maps, core_ids=list(range(8))).results

    out = np.empty((B, S, D), dtype=np.float32)
    for b in range(B):
        out[b] = res[2 * b]["out"] + res[2 * b + 1]["out"]
    return out


# revision 18
# speedup vs baseline: 1.1980x; 1.0371x over previous
"""Multi-head attention (B=4, S=2048, D=1024, H=16, dk=dv=64) on 8 TRN2 cores.

Sharding: core = (batch b, head-group g): data-parallel over batch (4) x
tensor-parallel over heads (2 groups of 8). Each core computes its batch's
Q/K/V projections for its 8 heads, attention, and a partial output
projection over its heads' rows of Wo. The host sums the two partial
outputs per batch.

Per-core kernel, all matmul operands bf16 (fp32 PSUM accumulate):
  xT is DMA'd once into SBUF (bf16, 32KB/partition) and reused by all
  projections. Heads are processed in 4 pairs; per pair Q/K are projected
  pair-packed ([2x64 dk, S]).

  Attention is a flat pipeline over (pair, query-block, key-chunk) steps
  paced by the ACT engine (exp of the 128x1024 score tile is the global
  floor at ~266us/core). Per key chunk both heads' scores land in one
  [128, 1024] PSUM tile and one exp covers both. The AV matmul is
  query-stationary: lhsT = exp-tile slice [128 s, 128 q], rhs =
  [V_h | ones] [128 s, 65] -> PSUM [128 q, 65] accumulated over 16 key
  chunks; column 64 is the softmax denominator (65 rows/matmul instead of
  512 for the value-stationary form). Only the first of the four
  query-chunk groups per PSUM bank passes start=True: start zeroes the
  whole 2KB bank, so the other groups bootstrap off the bank-wide
  pending-zero. After a block's 16 chunks the accumulator is copied to
  SBUF (freeing the bank), normalized with a DVE reciprocal +
  per-partition-scalar multiply, and PE-transposed back to [hv, q] for
  the output projection.

  All projection / V / output-projection matmul groups are emitted at a
  strongly deprioritized Tile priority: the list scheduler then slots
  them into PE gaps behind the attention stream instead of ahead of it,
  which keeps the ACT queue fed.
"""

import numpy as np

import concourse.bacc as bacc
import concourse.tile as tile
import concourse.mybir as mybir
from concourse.bass_utils import run_bass_kernel_spmd

F32 = mybir.dt.float32
BF16 = mybir.dt.bfloat16
EXP = mybir.ActivationFunctionType.Exp

P = 128
S = 2048
D = 1024
DK = 64
HPC = 8            # heads per core
NSC = S // P       # 16 key chunks of 128
NST = 4            # s-tiles of 512
STW = S // NST     # 512
NDC = D // P       # 8 d_model chunks
NPAIR = HPC // 2   # 4 head pairs
NQB = S // STW     # 4 query blocks of 512
NQC = STW // P     # 4 query chunks of 128 per block
SCALE = 1.0 / np.sqrt(DK)
LOWP = 10**7       # priority offset pushing filler behind the attention stream


def build_kernel():
    nc = bacc.Bacc("TRN2", target_bir_lowering=False, debug=False)

    xt_d = nc.dram_tensor("xt", [D, S], BF16, kind="ExternalInput")
    wq_d = nc.dram_tensor("wq", [NPAIR, D, P], BF16, kind="ExternalInput")
    wk_d = nc.dram_tensor("wk", [NPAIR, D, P], BF16, kind="ExternalInput")
    wv_d = nc.dram_tensor("wv", [D, HPC * DK], BF16, kind="ExternalInput")
    wo_d = nc.dram_tensor("wo", [HPC * DK, D], BF16, kind="ExternalInput")
    id_d = nc.dram_tensor("ident", [P, P], BF16, kind="ExternalInput")
    out_d = nc.dram_tensor("out", [S, D], F32, kind="ExternalOutput")

    xt_ap = xt_d.ap().rearrange("(dc p) s -> p dc s", p=P)

    with tile.TileContext(nc) as tc:
        with tc.tile_pool(name="persist", bufs=1) as persist, \
             tc.tile_pool(name="qkpool", bufs=2) as qkpool, \
             tc.tile_pool(name="wqkp", bufs=2) as wqkp, \
             tc.tile_pool(name="expp", bufs=6) as expp, \
             tc.tile_pool(name="avsbp", bufs=2) as avsbp, \
             tc.tile_pool(name="htsp", bufs=3) as htsp, \
             tc.tile_pool(name="recp", bufs=4) as recp, \
             tc.tile_pool(name="stage", bufs=6) as stage, \
             tc.tile_pool(name="scps", bufs=2, space="PSUM") as scps, \
             tc.tile_pool(name="avps", bufs=1, space="PSUM") as avps, \
             tc.tile_pool(name="qkps", bufs=2, space="PSUM") as qkps:

            xts = persist.tile([P, NDC, S], BF16, tag="xts")
            v520 = persist.tile([P, NSC, HPC, DK + 1], BF16, tag="v520")
            hn = persist.tile([P, NPAIR, S], BF16, tag="hn")
            wo_sb = persist.tile([P, NPAIR, D], BF16, tag="wo")
            wv_sb = persist.tile([P, NDC, HPC * DK], BF16, tag="wv")
            id_sb = persist.tile([P, P], BF16, tag="id")

            # ---------------- prologue DMAs ----------------
            wqk_tiles = {}

            def fetch_wqk(pr):
                wqp = wqkp.tile([P, NDC, P], BF16, tag="wqp", name=f"wqp{pr}")
                wkp = wqkp.tile([P, NDC, P], BF16, tag="wkp", name=f"wkp{pr}")
                nc.sync.dma_start(
                    wqp[:], wq_d.ap()[pr].rearrange("(dc p) c -> p dc c", p=P))
                nc.sync.dma_start(
                    wkp[:], wk_d.ap()[pr].rearrange("(dc p) c -> p dc c", p=P))
                wqk_tiles[pr] = (wqp, wkp)

            fetch_wqk(0)
            # xT in 16 chunks; query-block-0 columns first so pair-0 can
            # start; wv before the later chunks (V projection rides early),
            # ident/wo last (needed only from the first norm / pair 3).
            def xts_qtr(qtr):
                for dh in range(NDC // 2):
                    nc.sync.dma_start(
                        xts[:, 2 * dh:2 * dh + 2, qtr * 512:(qtr + 1) * 512],
                        xt_ap[:, 2 * dh:2 * dh + 2, qtr * 512:(qtr + 1) * 512])

            xts_qtr(0)
            xts_qtr(1)
            nc.sync.dma_start(
                wv_sb[:], wv_d.ap().rearrange("(dc p) c -> p dc c", p=P))
            for qtr in range(2, 4):
                xts_qtr(qtr)
            nc.sync.dma_start(id_sb[:], id_d.ap())
            nc.sync.dma_start(
                wo_sb[:], wo_d.ap().rearrange("(ci p) d -> p ci d", p=P))
            nc.vector.memset(v520[:, :, :, DK:DK + 1], 1.0)

            qk_tiles = {0: (qkpool.tile([P, S], BF16, tag="qt", name="qt0"),
                            qkpool.tile([P, S], BF16, tag="kt", name="kt0"))}

            # ---------------- helpers ----------------
            def qk_group(pr, which, st, sub_sc=False):
                """Project qtp/ktp columns st*512:(st+1)*512 for pair pr."""
                w_sb = wqk_tiles[pr][0 if which == "q" else 1]
                dst = qk_tiles[pr][0 if which == "q" else 1]
                ps = qkps.tile([P, STW], F32, tag="qkp", name=f"{which}{pr}{st}")
                if sub_sc:
                    # key-chunk granular psum + copies (fast prologue start)
                    for scl in range(4):
                        for dc in range(NDC):
                            nc.tensor.matmul(
                                ps[:, scl * P:(scl + 1) * P], w_sb[:, dc, :],
                                xts[:, dc, (st * 4 + scl) * P:(st * 4 + scl + 1) * P],
                                start=(dc == 0), stop=(dc == NDC - 1))
                        nc.vector.tensor_copy(
                            dst[:, (st * 4 + scl) * P:(st * 4 + scl + 1) * P],
                            ps[:, scl * P:(scl + 1) * P])
                else:
                    for dc in range(NDC):
                        nc.tensor.matmul(
                            ps[:], w_sb[:, dc, :],
                            xts[:, dc, st * STW:(st + 1) * STW],
                            start=(dc == 0), stop=(dc == NDC - 1))
                    nc.vector.tensor_copy(
                        dst[:, st * STW:(st + 1) * STW], ps[:])

            def v_group(pr, sc):
                """Project V for pair pr's two heads, key chunk sc."""
                ps = qkps.tile([P, STW], F32, tag="qkp", name=f"v{pr}{sc}")
                for dc in range(NDC):
                    nc.tensor.matmul(
                        ps[:, 0:P], xts[:, dc, sc * P:(sc + 1) * P],
                        wv_sb[:, dc, pr * P:(pr + 1) * P],
                        start=(dc == 0), stop=(dc == NDC - 1))
                nc.vector.tensor_copy(
                    v520[:, sc, 2 * pr:2 * pr + 2, 0:DK],
                    ps[:, 0:P].rearrange("p (h v) -> p h v", v=DK))

            def out_group(sc_out, dmh, pool=None):
                if pool is scps:
                    pa = pool.tile([P, 2 * STW], F32, tag="scp",
                                   name=f"o{sc_out}{dmh}")[:, 0:STW]
                else:
                    pa = qkps.tile([P, STW], F32, tag="qkp",
                                   name=f"o{sc_out}{dmh}")[:]
                for ci in range(NPAIR):
                    nc.tensor.matmul(
                        pa, hn[:, ci, sc_out * P:(sc_out + 1) * P],
                        wo_sb[:, ci, dmh * 512:(dmh + 1) * 512],
                        start=(ci == 0), stop=(ci == NPAIR - 1))
                osb = stage.tile([P, STW], F32, tag="ostage", name="osb")
                nc.vector.tensor_copy(osb[:], pa)
                nc.sync.dma_start(
                    out_d.ap()[sc_out * P:(sc_out + 1) * P,
                               dmh * 512:(dmh + 1) * 512],
                    osb[:])

            def scores_exp(pr, qb, sc):
                qtp, ktp = qk_tiles[pr]
                q0 = qb * STW
                scp = scps.tile([P, 2 * STW], F32, tag="scp", name="scp")
                for j in range(2):
                    nc.tensor.matmul(
                        scp[:, j * STW:(j + 1) * STW],
                        ktp[j * DK:(j + 1) * DK, sc * P:(sc + 1) * P],
                        qtp[j * DK:(j + 1) * DK, q0:q0 + STW],
                        start=True, stop=True)
                ex = expp.tile([P, 2 * STW], BF16, tag="ex", name="ex")
                nc.scalar.activation(ex[:], scp[:], EXP, scale=float(SCALE))
                return ex

            av_cur = [None]

            def av_mms(pr, qb, sc, ex):
                if sc == 0:
                    av_cur[0] = [avps.tile([P, NQC * 65], F32, tag=f"av{j}",
                                           name=f"av{j}_{pr}{qb}")
                                 for j in range(2)]
                av = av_cur[0]
                for j in range(2):
                    for qc in range(NQC):
                        nc.tensor.matmul(
                            av[j][:, qc * 65:(qc + 1) * 65],
                            ex[:, j * STW + qc * P:j * STW + (qc + 1) * P],
                            v520[:, sc, 2 * pr + j, :],
                            # start=True zeroes the whole 2KB psum bank, so
                            # only the first of the 4 interleaved qc groups
                            # starts; the rest overwrite-on-first-write via
                            # the bank-wide pending-zero.
                            start=(sc == 0 and qc == 0),
                            stop=(sc == NSC - 1),
                            skip_group_check=True)
                if sc == NSC - 1:
                    avsb = avsbp.tile([P, 2, NQC * 65], F32, tag="avsb",
                                      name=f"avsb{pr}{qb}")
                    for j in range(2):
                        nc.vector.tensor_copy(avsb[:, j, :], av[j][:])
                    return avsb
                return None

            def norm_qc(pr, qb, avsb, qc):
                """Normalize + transpose one 128-query chunk of both heads.

                The transpose lands in the (between-blocks idle) av0 PSUM
                bank rather than the qkp pool, keeping the qkp rotation
                free for projection / output-projection groups. Emission
                order guarantees the next block's AV matmuls (which re-start
                the bank) come after the hn copies below.
                """
                hts = htsp.tile([P, P], BF16, tag="hts", name=f"h{pr}{qb}{qc}")
                for j in range(2):
                    rec = recp.tile([P, 1], F32, tag="rec", name="rec")
                    nc.vector.reciprocal(
                        rec[:], avsb[:, j, qc * 65 + DK:qc * 65 + DK + 1])
                    nc.vector.tensor_scalar_mul(
                        hts[:, j * DK:(j + 1) * DK],
                        avsb[:, j, qc * 65:qc * 65 + DK], rec[:])
                tpb = av_cur[0][0][:, qc * DK:(qc + 1) * DK].bitcast(BF16)
                nc.tensor.transpose(tpb, hts[:], id_sb[:])
                nc.vector.tensor_copy(
                    hn[:, pr, (qb * NQC + qc) * P:(qb * NQC + qc + 1) * P],
                    tpb)

            # ---------------- prologue compute (pair 0 heads) --------------
            qk_group(0, "q", 0, sub_sc=True)
            qk_group(0, "k", 0, sub_sc=True)
            for sc in range(4):
                v_group(0, sc)
            with tc.high_priority(offset=-LOWP):
                for st in range(1, NST):
                    qk_group(0, "q", st)
                    qk_group(0, "k", st)
                for sc in range(4, NSC):
                    v_group(0, sc)

            # ---------------- flat attention pipeline ----------------
            steps = [(pr, qb, sc)
                     for pr in range(NPAIR)
                     for qb in range(NQB)
                     for sc in range(NSC)]
            exs = {}
            for t in range(len(steps) + 1):
                if t < len(steps):
                    cur = steps[t]
                    if cur[1:] == (0, 0):
                        pr0 = cur[0]
                        with tc.high_priority(offset=-LOWP):
                            if pr0 in (1, 2):
                                # this pair's late K columns / V chunks, only
                                # needed 8+ steps in: keep them in this
                                # pair's own span to unload the previous one
                                for st in range(2, NST):
                                    qk_group(pr0, "k", st)
                                for sc in range(NSC // 2, NSC):
                                    v_group(pr0, sc)
                        if pr0 + 1 < NPAIR:
                            # stage next pair's early projections as filler
                            # (pair 3 is staged fully here: its own span is
                            # loaded with the output-projection groups)
                            nxt = pr0 + 1
                            qk_tiles[nxt] = (
                                qkpool.tile([P, S], BF16, tag="qt",
                                            name=f"qt{nxt}"),
                                qkpool.tile([P, S], BF16, tag="kt",
                                            name=f"kt{nxt}"))
                            fetch_wqk(nxt)
                            with tc.high_priority(offset=-LOWP):
                                for st in range(NST):
                                    qk_group(nxt, "q", st)
                                k_hi = NST if nxt == NPAIR - 1 else 2
                                for st in range(k_hi):
                                    qk_group(nxt, "k", st)
                                v_hi = NSC if nxt == NPAIR - 1 else NSC // 2
                                for sc in range(v_hi):
                                    v_group(nxt, sc)
                    exs[cur] = scores_exp(*cur)
                if t > 0:
                    prv = steps[t - 1]
                    avsb = av_mms(*prv, exs.pop(prv))
                    if avsb is not None:
                        ppr, pqb = prv[0], prv[1]
                        for qc in range(NQC):
                            norm_qc(ppr, pqb, avsb, qc)
                            if ppr == NPAIR - 1:
                                so = pqb * NQC + qc
                                if pqb < NQB - 1:
                                    with tc.high_priority(offset=-LOWP):
                                        out_group(so, 0)
                                        out_group(so, 1)
                                else:
                                    # epilogue: scores are done, so borrow
                                    # the scp banks for a 4-deep rotation
                                    out_group(so, 0, pool=scps)
                                    out_group(so, 1)

    nc.compile()
    return nc


_NC_CACHE = None


def _get_nc():
    global _NC_CACHE
    if _NC_CACHE is None:
        _NC_CACHE = build_kernel()
    return _NC_CACHE


def kernel(x, Wq, Wk, Wv, Wo):
    import ml_dtypes
    bf16 = ml_dtypes.bfloat16

    x = np.asarray(x, dtype=np.float32)
    Wq = np.asarray(Wq, dtype=np.float32)
    Wk = np.asarray(Wk, dtype=np.float32)
    Wv = np.asarray(Wv, dtype=np.float32)
    Wo = np.asarray(Wo, dtype=np.float32)
    B = x.shape[0]
    ident = np.eye(P, dtype=bf16)

    in_maps = []
    for core in range(8):
        b, g = divmod(core, 2)
        hs = g * HPC
        xt = np.ascontiguousarray(x[b].T).astype(bf16)
        wq = np.stack([
            np.concatenate([Wq[hs + 2 * p], Wq[hs + 2 * p + 1]], axis=1)
            for p in range(NPAIR)]).astype(bf16)
        wk = np.stack([
            np.concatenate([Wk[hs + 2 * p], Wk[hs + 2 * p + 1]], axis=1)
            for p in range(NPAIR)]).astype(bf16)
        wv = np.concatenate([Wv[hs + h] for h in range(HPC)], axis=1).astype(bf16)
        wo = np.ascontiguousarray(Wo[hs * DK:(hs + HPC) * DK, :]).astype(bf16)
        in_maps.append({"xt": xt, "wq": wq, "wk": wk, "wv": wv, "wo": wo,
                        "ident": ident})

    nc = _get_nc()
    res = run_bass_kernel_spmd(nc, in_maps, core_ids=list(range(8))).results

    out = np.empty((B, S, D), dtype=np.float32)
    for b in range(B):
        out[b] = res[2 * b]["out"] + res[2 * b + 1]["out"]
    return out


# revision 22
# speedup vs baseline: 1.1988x; 1.0007x over previous
"""Multi-head attention (B=4, S=2048, D=1024, H=16, dk=dv=64) on 8 TRN2 cores.

Sharding: core = (batch b, head-group g): data-parallel over batch (4) x
tensor-parallel over heads (2 groups of 8). Each core computes its batch's
Q/K/V projections for its 8 heads, attention, and a partial output
projection over its heads' rows of Wo. The host sums the two partial
outputs per batch.

Per-core kernel, all matmul operands bf16 (fp32 PSUM accumulate):
  xT is DMA'd once into SBUF (bf16, 32KB/partition) and reused by all
  projections. Heads are processed in 4 pairs; per pair Q/K are projected
  pair-packed ([2x64 dk, S]).

  Attention is a flat pipeline over (pair, query-block, key-chunk) steps
  paced by the ACT engine (exp of the 128x1024 score tile is the global
  floor at ~266us/core). Per key chunk both heads' scores land in one
  [128, 1024] PSUM tile and one exp covers both. The AV matmul is
  query-stationary: lhsT = exp-tile slice [128 s, 128 q], rhs =
  [V_h | ones] [128 s, 65] -> PSUM [128 q, 65] accumulated over 16 key
  chunks; column 64 is the softmax denominator (65 rows/matmul instead of
  512 for the value-stationary form). Only the first of the four
  query-chunk groups per PSUM bank passes start=True: start zeroes the
  whole 2KB bank, so the other groups bootstrap off the bank-wide
  pending-zero. After a block's 16 chunks the accumulator is copied to
  SBUF (freeing the bank), normalized with a DVE reciprocal +
  per-partition-scalar multiply, and PE-transposed back to [hv, q] for
  the output projection.

  All projection / V / output-projection matmul groups are emitted at a
  strongly deprioritized Tile priority: the list scheduler then slots
  them into PE gaps behind the attention stream instead of ahead of it,
  which keeps the ACT queue fed.
"""

import numpy as np

import concourse.bacc as bacc
import concourse.tile as tile
import concourse.mybir as mybir
from concourse.bass_utils import run_bass_kernel_spmd

F32 = mybir.dt.float32
BF16 = mybir.dt.bfloat16
EXP = mybir.ActivationFunctionType.Exp

P = 128
S = 2048
D = 1024
DK = 64
HPC = 8            # heads per core
NSC = S // P       # 16 key chunks of 128
NST = 4            # s-tiles of 512
STW = S // NST     # 512
NDC = D // P       # 8 d_model chunks
NPAIR = HPC // 2   # 4 head pairs
NQB = S // STW     # 4 query blocks of 512
NQC = STW // P     # 4 query chunks of 128 per block
SCALE = 1.0 / np.sqrt(DK)
LOWP = 10**7       # priority offset pushing filler behind the attention stream


def build_kernel():
    nc = bacc.Bacc("TRN2", target_bir_lowering=False, debug=False)

    xt_d = nc.dram_tensor("xt", [D, S], BF16, kind="ExternalInput")
    wq_d = nc.dram_tensor("wq", [NPAIR, D, P], BF16, kind="ExternalInput")
    wk_d = nc.dram_tensor("wk", [NPAIR, D, P], BF16, kind="ExternalInput")
    wv_d = nc.dram_tensor("wv", [D, HPC * DK], BF16, kind="ExternalInput")
    wo_d = nc.dram_tensor("wo", [HPC * DK, D], BF16, kind="ExternalInput")
    id_d = nc.dram_tensor("ident", [P, P], BF16, kind="ExternalInput")
    out_d = nc.dram_tensor("out", [S, D], F32, kind="ExternalOutput")

    xt_ap = xt_d.ap().rearrange("(dc p) s -> p dc s", p=P)

    with tile.TileContext(nc) as tc:
        with tc.tile_pool(name="persist", bufs=1) as persist, \
             tc.tile_pool(name="qkpool", bufs=2) as qkpool, \
             tc.tile_pool(name="wqkp", bufs=2) as wqkp, \
             tc.tile_pool(name="expp", bufs=6) as expp, \
             tc.tile_pool(name="avsbp", bufs=2) as avsbp, \
             tc.tile_pool(name="htsp", bufs=3) as htsp, \
             tc.tile_pool(name="recp", bufs=4) as recp, \
             tc.tile_pool(name="stage", bufs=6) as stage, \
             tc.tile_pool(name="scps", bufs=2, space="PSUM") as scps, \
             tc.tile_pool(name="avps", bufs=1, space="PSUM") as avps, \
             tc.tile_pool(name="qkps", bufs=2, space="PSUM") as qkps:

            xts = persist.tile([P, NDC, S], BF16, tag="xts")
            v520 = persist.tile([P, NSC, HPC, DK + 1], BF16, tag="v520")
            hn = persist.tile([P, NPAIR, S], BF16, tag="hn")
            wo_sb = persist.tile([P, NPAIR, D], BF16, tag="wo")
            wv_sb = persist.tile([P, NDC, HPC * DK], BF16, tag="wv")
            id_sb = persist.tile([P, P], BF16, tag="id")

            # ---------------- prologue DMAs ----------------
            wqk_tiles = {}

            def fetch_wqk(pr):
                wqp = wqkp.tile([P, NDC, P], BF16, tag="wqp", name=f"wqp{pr}")
                wkp = wqkp.tile([P, NDC, P], BF16, tag="wkp", name=f"wkp{pr}")
                nc.sync.dma_start(
                    wqp[:], wq_d.ap()[pr].rearrange("(dc p) c -> p dc c", p=P))
                nc.sync.dma_start(
                    wkp[:], wk_d.ap()[pr].rearrange("(dc p) c -> p dc c", p=P))
                wqk_tiles[pr] = (wqp, wkp)

            fetch_wqk(0)
            # xT in 16 chunks; query-block-0 columns first so pair-0 can
            # start; wv before the later chunks (V projection rides early),
            # ident/wo last (needed only from the first norm / pair 3).
            def xts_qtr(qtr):
                for dh in range(NDC // 2):
                    nc.sync.dma_start(
                        xts[:, 2 * dh:2 * dh + 2, qtr * 512:(qtr + 1) * 512],
                        xt_ap[:, 2 * dh:2 * dh + 2, qtr * 512:(qtr + 1) * 512])

            xts_qtr(0)
            xts_qtr(1)
            nc.sync.dma_start(
                wv_sb[:], wv_d.ap().rearrange("(dc p) c -> p dc c", p=P))
            for qtr in range(2, 4):
                xts_qtr(qtr)
            nc.sync.dma_start(id_sb[:], id_d.ap())
            nc.sync.dma_start(
                wo_sb[:], wo_d.ap().rearrange("(ci p) d -> p ci d", p=P))
            nc.vector.memset(v520[:, :, :, DK:DK + 1], 1.0)

            qk_tiles = {0: (qkpool.tile([P, S], BF16, tag="qt", name="qt0"),
                            qkpool.tile([P, S], BF16, tag="kt", name="kt0"))}

            # ~3.4us of dummy matmuls on a zero scratch tile: the PE is
            # waiting on the prologue DMAs anyway, and this lifts it out of
            # the mid p-state before the first real projection group.
            scratch = persist.tile([P, STW], BF16, tag="scratch")
            nc.vector.memset(scratch[:], 0.0)
            for i in range(8):
                wps = qkps.tile([P, STW], F32, tag="qkp", name=f"warm{i}")
                nc.tensor.matmul(wps[:], scratch[:, 0:P], scratch[:],
                                 start=True, stop=True)

            # ---------------- helpers ----------------
            def qk_group(pr, which, st, sub_sc=False):
                """Project qtp/ktp columns st*512:(st+1)*512 for pair pr."""
                w_sb = wqk_tiles[pr][0 if which == "q" else 1]
                dst = qk_tiles[pr][0 if which == "q" else 1]
                ps = qkps.tile([P, STW], F32, tag="qkp", name=f"{which}{pr}{st}")
                if sub_sc:
                    # key-chunk granular psum + copies (fast prologue start)
                    for scl in range(4):
                        for dc in range(NDC):
                            nc.tensor.matmul(
                                ps[:, scl * P:(scl + 1) * P], w_sb[:, dc, :],
                                xts[:, dc, (st * 4 + scl) * P:(st * 4 + scl + 1) * P],
                                start=(dc == 0), stop=(dc == NDC - 1))
                        nc.vector.tensor_copy(
                            dst[:, (st * 4 + scl) * P:(st * 4 + scl + 1) * P],
                            ps[:, scl * P:(scl + 1) * P])
                else:
                    for dc in range(NDC):
                        nc.tensor.matmul(
                            ps[:], w_sb[:, dc, :],
                            xts[:, dc, st * STW:(st + 1) * STW],
                            start=(dc == 0), stop=(dc == NDC - 1))
                    nc.vector.tensor_copy(
                        dst[:, st * STW:(st + 1) * STW], ps[:])

            def v_group(pr, sc):
                """Project V for pair pr's two heads, key chunk sc."""
                ps = qkps.tile([P, STW], F32, tag="qkp", name=f"v{pr}{sc}")
                for dc in range(NDC):
                    nc.tensor.matmul(
                        ps[:, 0:P], xts[:, dc, sc * P:(sc + 1) * P],
                        wv_sb[:, dc, pr * P:(pr + 1) * P],
                        start=(dc == 0), stop=(dc == NDC - 1))
                nc.vector.tensor_copy(
                    v520[:, sc, 2 * pr:2 * pr + 2, 0:DK],
                    ps[:, 0:P].rearrange("p (h v) -> p h v", v=DK))

            def out_group(sc_out, dmh, pool=None):
                if pool is scps:
                    pa = pool.tile([P, 2 * STW], F32, tag="scp",
                                   name=f"o{sc_out}{dmh}")[:, 0:STW]
                else:
                    pa = qkps.tile([P, STW], F32, tag="qkp",
                                   name=f"o{sc_out}{dmh}")[:]
                for ci in range(NPAIR):
                    nc.tensor.matmul(
                        pa, hn[:, ci, sc_out * P:(sc_out + 1) * P],
                        wo_sb[:, ci, dmh * 512:(dmh + 1) * 512],
                        start=(ci == 0), stop=(ci == NPAIR - 1))
                osb = stage.tile([P, STW], F32, tag="ostage", name="osb")
                nc.vector.tensor_copy(osb[:], pa)
                nc.sync.dma_start(
                    out_d.ap()[sc_out * P:(sc_out + 1) * P,
                               dmh * 512:(dmh + 1) * 512],
                    osb[:])

            def scores_exp(pr, qb, sc):
                qtp, ktp = qk_tiles[pr]
                q0 = qb * STW
                scp = scps.tile([P, 2 * STW], F32, tag="scp", name="scp")
                for j in range(2):
                    nc.tensor.matmul(
                        scp[:, j * STW:(j + 1) * STW],
                        ktp[j * DK:(j + 1) * DK, sc * P:(sc + 1) * P],
                        qtp[j * DK:(j + 1) * DK, q0:q0 + STW],
                        start=True, stop=True)
                ex = expp.tile([P, 2 * STW], BF16, tag="ex", name="ex")
                nc.scalar.activation(ex[:], scp[:], EXP, scale=float(SCALE))
                return ex

            av_cur = [None]

            def av_mms(pr, qb, sc, ex):
                if sc == 0:
                    av_cur[0] = [avps.tile([P, NQC * 65], F32, tag=f"av{j}",
                                           name=f"av{j}_{pr}{qb}")
                                 for j in range(2)]
                av = av_cur[0]
                for j in range(2):
                    for qc in range(NQC):
                        nc.tensor.matmul(
                            av[j][:, qc * 65:(qc + 1) * 65],
                            ex[:, j * STW + qc * P:j * STW + (qc + 1) * P],
                            v520[:, sc, 2 * pr + j, :],
                            # start=True zeroes the whole 2KB psum bank, so
                            # only the first of the 4 interleaved qc groups
                            # starts; the rest overwrite-on-first-write via
                            # the bank-wide pending-zero.
                            start=(sc == 0 and qc == 0),
                            stop=(sc == NSC - 1),
                            skip_group_check=True)
                if sc == NSC - 1:
                    avsb = avsbp.tile([P, 2, NQC * 65], F32, tag="avsb",
                                      name=f"avsb{pr}{qb}")
                    for j in range(2):
                        nc.vector.tensor_copy(avsb[:, j, :], av[j][:])
                    return avsb
                return None

            def norm_qc(pr, qb, avsb, qc):
                """Normalize + transpose one 128-query chunk of both heads.

                The transpose lands in the (between-blocks idle) av0 PSUM
                bank rather than the qkp pool, keeping the qkp rotation
                free for projection / output-projection groups. Emission
                order guarantees the next block's AV matmuls (which re-start
                the bank) come after the hn copies below.
                """
                hts = htsp.tile([P, P], BF16, tag="hts", name=f"h{pr}{qb}{qc}")
                for j in range(2):
                    rec = recp.tile([P, 1], F32, tag="rec", name="rec")
                    nc.vector.reciprocal(
                        rec[:], avsb[:, j, qc * 65 + DK:qc * 65 + DK + 1])
                    nc.vector.tensor_scalar_mul(
                        hts[:, j * DK:(j + 1) * DK],
                        avsb[:, j, qc * 65:qc * 65 + DK], rec[:])
                tpb = av_cur[0][0][:, qc * DK:(qc + 1) * DK].bitcast(BF16)
                nc.tensor.transpose(tpb, hts[:], id_sb[:])
                nc.vector.tensor_copy(
                    hn[:, pr, (qb * NQC + qc) * P:(qb * NQC + qc + 1) * P],
                    tpb)

            # ---------------- prologue compute (pair 0 heads) --------------
            qk_group(0, "q", 0, sub_sc=True)
            qk_group(0, "k", 0, sub_sc=True)
            for sc in range(4):
                v_group(0, sc)
            with tc.high_priority(offset=-LOWP):
                for st in range(1, NST):
                    qk_group(0, "q", st)
                    qk_group(0, "k", st)
                for sc in range(4, NSC):
                    v_group(0, sc)

            # ---------------- flat attention pipeline ----------------
            steps = [(pr, qb, sc)
                     for pr in range(NPAIR)
                     for qb in range(NQB)
                     for sc in range(NSC)]
            exs = {}
            for t in range(len(steps) + 1):
                if t < len(steps):
                    cur = steps[t]
                    if cur[1:] == (0, 0):
                        pr0 = cur[0]
                        with tc.high_priority(offset=-LOWP):
                            if pr0 in (1, 2):
                                # this pair's late K columns / V chunks, only
                                # needed 8+ steps in: keep them in this
                                # pair's own span to unload the previous one
                                for st in range(2, NST):
                                    qk_group(pr0, "k", st)
                                for sc in range(NSC // 2, NSC):
                                    v_group(pr0, sc)
                        if pr0 + 1 < NPAIR:
                            # stage next pair's early projections as filler
                            # (pair 3 is staged fully here: its own span is
                            # loaded with the output-projection groups)
                            nxt = pr0 + 1
                            qk_tiles[nxt] = (
                                qkpool.tile([P, S], BF16, tag="qt",
                                            name=f"qt{nxt}"),
                                qkpool.tile([P, S], BF16, tag="kt",
                                            name=f"kt{nxt}"))
                            fetch_wqk(nxt)
                            with tc.high_priority(offset=-LOWP):
                                for st in range(NST):
                                    qk_group(nxt, "q", st)
                                k_hi = NST if nxt == NPAIR - 1 else 2
                                for st in range(k_hi):
                                    qk_group(nxt, "k", st)
                                v_hi = NSC if nxt == NPAIR - 1 else NSC // 2
                                for sc in range(v_hi):
                                    v_group(nxt, sc)
                    exs[cur] = scores_exp(*cur)
                if t > 0:
                    prv = steps[t - 1]
                    avsb = av_mms(*prv, exs.pop(prv))
                    if avsb is not None:
                        ppr, pqb = prv[0], prv[1]
                        for qc in range(NQC):
                            norm_qc(ppr, pqb, avsb, qc)
                            if ppr == NPAIR - 1:
                                so = pqb * NQC + qc
                                if pqb < NQB - 1:
                                    with tc.high_priority(offset=-LOWP):
                                        out_group(so, 0)
                                        out_group(so, 1)
                                else:
                                    # epilogue: scores are done, so borrow
                                    # the scp banks for a 4-deep rotation
                                    out_group(so, 0, pool=scps)
                                    out_group(so, 1)

    nc.compile()
    return nc


_NC_CACHE = None


def _get_nc():
    global _NC_CACHE
    if _NC_CACHE is None:
        _NC_CACHE = build_kernel()
    return _NC_CACHE


def kernel(x, Wq, Wk, Wv, Wo):
    import ml_dtypes
    bf16 = ml_dtypes.bfloat16

    x = np.asarray(x, dtype=np.float32)
    Wq = np.asarray(Wq, dtype=np.float32)
    Wk = np.asarray(Wk, dtype=np.float32)
    Wv = np.asarray(Wv, dtype=np.float32)
    Wo = np.asarray(Wo, dtype=np.float32)
    B = x.shape[0]
    ident = np.eye(P, dtype=bf16)

    in_maps = []
    for core in range(8):
        b, g = divmod(core, 2)
        hs = g * HPC
        xt = np.ascontiguousarray(x[b].T).astype(bf16)
        wq = np.stack([
            np.concatenate([Wq[hs + 2 * p], Wq[hs + 2 * p + 1]], axis=1)
            for p in range(NPAIR)]).astype(bf16)
        wk = np.stack([
            np.concatenate([Wk[hs + 2 * p], Wk[hs + 2 * p + 1]], axis=1)
            for p in range(NPAIR)]).astype(bf16)
        wv = np.concatenate([Wv[hs + h] for h in range(HPC)], axis=1).astype(bf16)
        wo = np.ascontiguousarray(Wo[hs * DK:(hs + HPC) * DK, :]).astype(bf16)
        in_maps.append({"xt": xt, "wq": wq, "wk": wk, "wv": wv, "wo": wo,
                        "ident": ident})

    nc = _get_nc()
    res = run_bass_kernel_spmd(nc, in_maps, core_ids=list(range(8))).results

    out = np.empty((B, S, D), dtype=np.float32)
    for b in range(B):
        out[b] = res[2 * b]["out"] + res[2 * b + 1]["out"]
    return out
